# revision 9
# baseline (speedup 1.0000x reference)
"""Trainium2 Bass kernel for nn_EmergentRiskMetrics.

Contract: kernel(**inputs) takes the FULL unsharded inputs (as produced by
setup_inputs()) and returns the FULL output (shape [8], float32).

Sharding: data-parallel over the time axis. Each of the 8 cores owns 1024
contiguous rows (plus a window halo for the rolling scans). The full-T
[A,A] covariance is computed as per-core partials (8 bf16 matmuls on the
core's own rows) combined with a single 64KB AllReduce that overlaps the
rolling-window compute; the nonlinear [A,A] post-processing (correlation,
|corr| mean, top-eigenvalue power chain) and the tiny MLP are replicated.
The sign-concordance sum needs no matmul at all: sum_ab (S^T S)_ab =
sum_t (sum_a sign(x_ta))^2, which shards linearly over t.

Device outputs are per-core scalars; the host only gathers them (sums the
sharded partials, applies final scalar clips/divides) into the 8 outputs.
"""

import numpy as np

T = 8192
A = 128
W20 = 20
W10 = 10
NC_N = 8
CHUNK = 1024            # window starts / owned rows per core
XROWS = 1152            # rolling-scan rows incl. halo (9 x 128)
R20 = 128 + W20 - 1     # 147
R10 = 128 + W10 - 1     # 137
N20 = T - W20           # 8172 rolling-20 windows
N10 = T - W10           # 8182 rolling-10 windows
OUT_SLOTS = 16
INV_OD = 1.0 / (A * (A - 1))
# rolling > 0.7 in terms of the unnormalized window sum y:
# rolling = (y - A) * INV_OD  =>  y > 0.7 * A*(A-1) + A
Y_THRESH = 0.7 * A * (A - 1) + A

# out_vec slot layout
(S_COUNT20, S_HIST10, S_RECENT10, S_CSSUM, S_CSFIRST, S_CSLAST,
 S_SUMCORR, S_SUMABS, S_TRACE, S_PASUM, S_PAMAX, S_SEV, S_SSQ,
 S_T7, S_T8) = range(15)

# packed-constant column layout (cmb: [128, CMB_W] f32)
M20_OFF = 0
M10_OFF = M20_OFF + R20          # 147
V20_OFF = M10_OFF + R10          # 284
H10_OFF = V20_OFF + 8            # 292
R10M_OFF = H10_OFF + 8           # 300
ID_OFF = R10M_OFF + 8            # 308
W1A_OFF = ID_OFF + 128           # 436
W1B_OFF = W1A_OFF + 128          # 564
W2_OFF = W1B_OFF + 128           # 692
B1_COL = W2_OFF + 64             # 756
GAM_COL = B1_COL + 1             # 757
BET_COL = GAM_COL + 1            # 758
POS_COL = BET_COL + 1            # 759
XL_COL = POS_COL + 1             # 760
OH127_COL = XL_COL + 1           # 761
W3_OFF = OH127_COL + 1           # 762 (3 cols, rows 0:64)
B2_COL = W3_OFF + 3              # 765 (rows 0:64)
B3_COL = B2_COL + 1              # 766 (rows 0:3)
OH2_COL = B3_COL + 1             # 767 (rows 0:3)
CMB_W = OH2_COL + 1              # 768

_PLAN = {}


def _build_program():
    import concourse.bacc as bacc
    import concourse.tile as tile
    from concourse import mybir

    f32 = mybir.dt.float32
    bf16 = mybir.dt.bfloat16
    ALU = mybir.AluOpType
    ACT = mybir.ActivationFunctionType
    AX = mybir.AxisListType

    nc = bacc.Bacc("TRN2", target_bir_lowering=False, debug=False,
                   num_devices=NC_N)

    def din(name, shape):
        return nc.dram_tensor(name, shape, f32, kind="ExternalInput").ap()

    x_chunk = din("x_chunk", [128, CHUNK])     # row blocks: [p, j*128+a]
    xT_in = din("xT_chunk", [128, XROWS])      # [a, t_local]
    cmb_in = din("cmb", [128, CMB_W])
    out_d = nc.dram_tensor("out_vec", [1, OUT_SLOTS], f32,
                           kind="ExternalOutput").ap()

    with tile.TileContext(nc) as tc:
        with tc.tile_pool(name="const", bufs=1) as cst, \
             tc.tile_pool(name="persist", bufs=1) as per, \
             tc.tile_pool(name="work", bufs=3) as wrk, \
             tc.tile_pool(name="scr", bufs=4) as scp, \
             tc.tile_pool(name="small", bufs=6) as sml, \
             tc.tile_pool(name="dram", bufs=2, space="DRAM") as dram, \
             tc.tile_pool(name="ps", bufs=1, space="PSUM") as ps:

            psum_bufs = {"zp": 2, "big": 2, "sc": 2, "r2p": 2}

            def psum(shape, tag, dtype=f32):
                return ps.tile(shape, dtype, tag=tag, name=tag,
                               bufs=psum_bufs[tag])

            # ---- inputs ----
            xc = cst.tile([128, CHUNK], f32, tag="xc")
            nc.sync.dma_start(xc[:], x_chunk[:, :])
            xT = cst.tile([128, XROWS], f32, tag="xT")
            nc.sync.dma_start(xT[:], xT_in[:, :])
            cmb = cst.tile([128, CMB_W], f32, tag="cmb")
            nc.sync.dma_start(cmb[:], cmb_in[:, :])

            m20 = cmb[:, M20_OFF:M20_OFF + R20]
            m10 = cmb[:, M10_OFF:M10_OFF + R10]
            v20 = cmb[:, V20_OFF:V20_OFF + 8]
            h10 = cmb[:, H10_OFF:H10_OFF + 8]
            r10m = cmb[:, R10M_OFF:R10M_OFF + 8]
            ident = cmb[:, ID_OFF:ID_OFF + 128]

            ones = cst.tile([128, 1], f32, tag="ones")
            nc.vector.memset(ones[:], 1.0)
            onesb = cst.tile([128, 1], bf16, tag="onesb")
            nc.vector.memset(onesb[:], 1.0)
            mb20 = cst.tile([128, R20], bf16, tag="mb20")
            nc.scalar.activation(mb20[:], cmb[:, M20_OFF:M20_OFF + R20],
                                 ACT.Copy)
            mb10 = cst.tile([128, R10], bf16, tag="mb10")
            nc.scalar.activation(mb10[:], cmb[:, M10_OFF:M10_OFF + R10],
                                 ACT.Copy)

            out_sb = per.tile([1, OUT_SLOTS], f32, tag="out_sb")
            nc.vector.memset(out_sb[:], 0.0)

            def slot(i):
                return out_sb[:, i:i + 1]

            # sum over partitions of an SBUF [p,1] vector -> [1,1] psum
            def psum_scalar(vec_sb, p=128):
                o = psum([1, 1], "sc")
                lhs = ones[0:p, :] if p != 128 else ones[:]
                nc.tensor.matmul(o[:], lhsT=lhs, rhs=vec_sb,
                                 start=True, stop=True, skip_group_check=True)
                return o

            # ============ Phase A: partial covariance -> AllReduce ============
            # bf16 copy of the row blocks; accum gives per-t cross-asset sums
            xcb = per.tile([128, CHUNK], bf16, tag="xcb")
            sr = per.tile([128, 8], f32, tag="sr")
            for j in range(8):
                nc.scalar.activation(xcb[:, j * 128:(j + 1) * 128],
                                     xc[:, j * 128:(j + 1) * 128],
                                     ACT.Copy, accum_out=sr[:, j:j + 1])
            covp = psum([128, 128], "big")
            for j in range(8):
                blk = xcb[:, j * 128:(j + 1) * 128]
                nc.tensor.matmul(covp[:], lhsT=blk, rhs=blk,
                                 start=(j == 0), stop=(j == 7),
                                 skip_group_check=True)
            cov_part = per.tile([128, 128], f32, tag="cov_part")
            nc.vector.tensor_copy(cov_part[:], covp[:])
            ar_in = dram.tile([128, 128], f32)
            ar_out = dram.tile([128, 128], f32)
            nc.sync.dma_start(ar_in[:], cov_part[:])
            nc.gpsimd.collective_compute(
                "AllReduce", ALU.add,
                replica_groups=[list(range(NC_N))],
                ins=[ar_in.opt()], outs=[ar_out.opt()])

            # ============ Phase B: row stats (csstd, sign concordance) =======
            ss = per.tile([128, 8], f32, tag="ss")
            rho = per.tile([128, 8], f32, tag="rho")
            for j in range(8):
                scr = scp.tile([128, 128], bf16, tag="scrB")
                nc.scalar.activation(scr[:], xc[:, j * 128:(j + 1) * 128],
                                     ACT.Square, accum_out=ss[:, j:j + 1])
            for j in range(8):
                scr = scp.tile([128, 128], bf16, tag="scrB")
                nc.scalar.activation(scr[:], xc[:, j * 128:(j + 1) * 128],
                                     ACT.Sign, accum_out=rho[:, j:j + 1])

            # cross-sectional std per t: sqrt((ss - sr^2/A) / (A-1))
            sq2 = sml.tile([128, 8], f32, tag="sq2")
            nc.scalar.activation(sq2[:], sr[:], ACT.Square,
                                 scale=float(1.0 / np.sqrt(A)))
            varA = sml.tile([128, 8], f32, tag="varA")
            nc.vector.scalar_tensor_tensor(
                varA[:], in0=sq2[:], scalar=-1.0, in1=ss[:],
                op0=ALU.mult, op1=ALU.add)
            csstd = per.tile([128, 8], f32, tag="csstd")
            nc.scalar.activation(csstd[:], varA[:], ACT.Sqrt,
                                 scale=float(1.0 / (A - 1)))
            csr = sml.tile([128, 1], f32, tag="csr")
            nc.vector.tensor_reduce(csr[:], csstd[:], axis=AX.X, op=ALU.add)
            nc.vector.tensor_copy(slot(S_CSSUM), psum_scalar(csr[:])[:])
            nc.vector.tensor_copy(slot(S_CSFIRST), csstd[0:1, 0:1])
            cslast_p = psum([1, 1], "sc")
            nc.tensor.matmul(cslast_p[:], lhsT=cmb[:, OH127_COL:OH127_COL + 1],
                             rhs=csstd[:, 7:8],
                             start=True, stop=True, skip_group_check=True)
            nc.vector.tensor_copy(slot(S_CSLAST), cslast_p[:])

            # ssq partial: sum_t rho_t^2
            rr = sml.tile([128, 1], f32, tag="rr")
            rho2 = sml.tile([128, 8], f32, tag="rho2")
            nc.scalar.activation(rho2[:], rho[:], ACT.Square, accum_out=rr[:])
            nc.vector.tensor_copy(slot(S_SSQ), psum_scalar(rr[:])[:])

            # ============ Phase C: rolling windows ============
            xTb = per.tile([128, XROWS], bf16, tag="xTb")
            nc.scalar.activation(xTb[:], xT[:], ACT.Copy)
            x2T = per.tile([128, 1151], bf16, tag="x2T")
            nc.scalar.activation(x2T[:], xT[:, 0:1151], ACT.Square)

            # rolling sums via log-shift adds (S on vector, P=sum x^2 on gpsimd)
            def rollsums(src, tag, eng, w0):
                s2 = per.tile([128, w0], bf16, tag=tag + "s2")
                eng.tensor_add(s2[:], src[:, 0:w0], src[:, 1:w0 + 1])
                s4 = per.tile([128, w0 - 2], bf16, tag=tag + "s4")
                eng.tensor_add(s4[:], s2[:, 0:w0 - 2], s2[:, 2:w0])
                s8 = per.tile([128, w0 - 6], bf16, tag=tag + "s8")
                eng.tensor_add(s8[:], s4[:, 0:w0 - 6], s4[:, 4:w0 - 2])
                s16 = per.tile([128, w0 - 14], bf16, tag=tag + "s16")
                eng.tensor_add(s16[:], s8[:, 0:w0 - 14], s8[:, 8:w0 - 6])
                s20 = per.tile([128, CHUNK], bf16, tag=tag + "s20")
                eng.tensor_add(s20[:], s16[:, 0:CHUNK], s4[:, 16:CHUNK + 16])
                s10 = per.tile([128, CHUNK], bf16, tag=tag + "s10")
                eng.tensor_add(s10[:], s8[:, 0:CHUNK], s2[:, 8:CHUNK + 8])
                return s20, s10

            S20, S10 = rollsums(xTb, "S", nc.vector, 1151)
            P20, P10 = rollsums(x2T, "P", nc.gpsimd, 1150)

            # u = 1/sqrt(P - S^2/w), bf16 for the z-matmul lhsT
            def make_u(S, P, w, tag):
                ssq_ = per.tile([128, CHUNK], bf16, tag=tag + "ssq")
                nc.scalar.activation(ssq_[:], S[:], ACT.Square,
                                     scale=float(1.0 / np.sqrt(w)))
                d2 = per.tile([128, CHUNK], f32, tag=tag + "d2")
                nc.gpsimd.tensor_tensor(d2[:], P[:], ssq_[:], ALU.subtract)
                rcp = per.tile([128, CHUNK], f32, tag=tag + "rcp")
                nc.vector.reciprocal_approx_fast(rcp[:], d2[:])
                u = per.tile([128, CHUNK], bf16, tag=tag + "u")
                nc.scalar.activation(u[:], rcp[:], ACT.Sqrt)
                return u

            u20 = make_u(S20, P20, W20, "u20")
            u10 = make_u(S10, P10, W10, "u10")
            us20 = per.tile([128, CHUNK], bf16, tag="us20")
            nc.gpsimd.tensor_mul(us20[:], u20[:], S20[:])
            us10 = per.tile([128, CHUNK], bf16, tag="us10")
            nc.gpsimd.tensor_mul(us10[:], u10[:], S10[:])

            # per-chunk window sums y = r1 - r2^2/w  (rolling = (y-A)*inv_od)
            y20 = per.tile([128, 8], f32, tag="y20")
            y10 = per.tile([128, 8], f32, tag="y10")

            def roll_chunk(k, u, us, R, mb, w, ycol):
                ksl = slice(k * 128, (k + 1) * 128)
                zp = psum([128, R20], "zp")
                nc.tensor.matmul(zp[:, 0:R], lhsT=u[:, ksl],
                                 rhs=xTb[:, k * 128:k * 128 + R],
                                 start=True, stop=True, skip_group_check=True)
                r2p = psum([128, 1], "r2p")
                nc.tensor.matmul(r2p[:], lhsT=us[:, ksl], rhs=onesb[:],
                                 start=True, stop=True, skip_group_check=True)
                V = wrk.tile([128, R20], bf16, tag="V")
                nc.scalar.activation(V[:, 0:R], zp[:, 0:R], ACT.Square)
                rsqw = sml.tile([128, 1], f32, tag="rsqw")
                nc.scalar.activation(rsqw[:], r2p[:], ACT.Square,
                                     scale=float(1.0 / np.sqrt(w)))
                Vm = wrk.tile([128, R20], bf16, tag="Vm")
                nc.vector.tensor_mul(Vm[:, 0:R], V[:, 0:R], mb)
                r1 = sml.tile([128, 1], f32, tag="r1")
                nc.vector.tensor_reduce(r1[:], Vm[:, 0:R], axis=AX.X,
                                        op=ALU.add)
                nc.vector.scalar_tensor_tensor(
                    ycol, in0=rsqw[:], scalar=-1.0, in1=r1[:],
                    op0=ALU.mult, op1=ALU.add)

            for k in range(8):
                roll_chunk(k, u20, us20, R20, mb20, W20, y20[:, k:k + 1])
                roll_chunk(k, u10, us10, R10, mb10, W10, y10[:, k:k + 1])

            # batched finals
            c1 = sml.tile([128, 8], f32, tag="c1")
            nc.vector.tensor_scalar(c1[:], y20[:], float(Y_THRESH), None,
                                    ALU.is_gt)
            c2 = sml.tile([128, 8], f32, tag="c2")
            nc.vector.tensor_mul(c2[:], c1[:], v20)
            cnt = sml.tile([128, 1], f32, tag="cnt")
            nc.vector.tensor_reduce(cnt[:], c2[:], axis=AX.X, op=ALU.add)
            nc.vector.tensor_copy(slot(S_COUNT20), psum_scalar(cnt[:])[:])
            hscr = sml.tile([128, 8], f32, tag="hscr")
            nc.vector.tensor_mul(hscr[:], y10[:], h10)
            hs = sml.tile([128, 1], f32, tag="hs")
            nc.vector.tensor_reduce(hs[:], hscr[:], axis=AX.X, op=ALU.add)
            nc.vector.tensor_copy(slot(S_HIST10), psum_scalar(hs[:])[:])
            rscr = sml.tile([128, 8], f32, tag="rscr")
            nc.vector.tensor_mul(rscr[:], y10[:], r10m)
            rs = sml.tile([128, 1], f32, tag="rs")
            nc.vector.tensor_reduce(rs[:], rscr[:], axis=AX.X, op=ALU.add)
            nc.vector.tensor_copy(slot(S_RECENT10), psum_scalar(rs[:])[:])

            # ============ Phase D: post-AllReduce [A,A] work ============
            cov = per.tile([128, 128], f32, tag="cov")
            nc.gpsimd.dma_start(cov[:], ar_out[:])

            dscr = wrk.tile([128, 128], f32, tag="dscr")
            nc.vector.tensor_mul(dscr[:], cov[:], ident)
            diag = sml.tile([128, 1], f32, tag="diag")
            nc.vector.tensor_reduce(diag[:], dscr[:], axis=AX.X, op=ALU.add)
            dstd = sml.tile([128, 1], f32, tag="dstd")
            nc.scalar.activation(dstd[:], diag[:], ACT.Sqrt)
            ucol = per.tile([128, 1], f32, tag="ucol")
            nc.vector.reciprocal(ucol[:], dstd[:])
            u2 = sml.tile([128, 1], f32, tag="u2")
            nc.vector.tensor_mul(u2[:], ucol[:], ucol[:])
            du2 = sml.tile([128, 1], f32, tag="du2")
            nc.vector.tensor_mul(du2[:], u2[:], diag[:])
            nc.vector.tensor_copy(slot(S_TRACE), psum_scalar(du2[:])[:])

            uT_p = psum([1, 128], "sc")
            nc.tensor.transpose(uT_p[:], ucol[:], ident)
            uT = per.tile([1, 128], f32, tag="uT")
            nc.vector.tensor_copy(uT[:], uT_p[:])

            def quad_form(mat_sb, out_slot):
                qr = psum([1, 128], "sc")
                nc.tensor.matmul(qr[:], lhsT=ucol[:], rhs=mat_sb,
                                 start=True, stop=True, skip_group_check=True)
                qscr = sml.tile([1, 128], f32, tag="qscr")
                nc.vector.tensor_mul(qscr[:], qr[:], uT[:])
                qacc = sml.tile([1, 1], f32, tag="qacc")
                nc.vector.tensor_reduce(qacc[:], qscr[:], axis=AX.X,
                                        op=ALU.add)
                nc.vector.tensor_copy(out_slot, qacc[:])

            quad_form(cov[:], slot(S_SUMCORR))
            acov = wrk.tile([128, 128], f32, tag="acov")
            nc.scalar.activation(acov[:], cov[:], ACT.Abs)
            quad_form(acov[:], slot(S_SUMABS))

            # corr in bf16: diag(u) cov diag(u) via row-scale, transpose, scale
            brow = wrk.tile([128, 128], f32, tag="brow")
            nc.vector.tensor_scalar(brow[:], cov[:], ucol[:], None, ALU.mult)
            bt_p = psum([128, 128], "big")
            nc.tensor.transpose(bt_p[:], brow[:], ident)
            corrb = per.tile([128, 128], bf16, tag="corrb")
            nc.vector.tensor_scalar(corrb[:], bt_p[:], ucol[:], None, ALU.mult)

            # top eigenvalue: 8 bf16 squarings; traces of M^128 and M^256
            M = corrb
            for kk in range(8):
                p = psum([128, 128], "big")
                nc.tensor.matmul(p[:], lhsT=M[:], rhs=M[:],
                                 start=True, stop=True, skip_group_check=True)
                if kk >= 6:
                    escr = wrk.tile([128, 128], f32, tag="escr")
                    nc.vector.tensor_mul(escr[:], p[:], ident)
                    edg = sml.tile([128, 1], f32, tag="edg")
                    nc.vector.tensor_reduce(edg[:], escr[:], axis=AX.X,
                                            op=ALU.add)
                    s = S_T7 if kk == 6 else S_T8
                    nc.vector.tensor_copy(slot(s), psum_scalar(edg[:])[:])
                if kk < 7:
                    Mn = wrk.tile([128, 128], bf16, tag="Mn", bufs=2)
                    nc.vector.tensor_copy(Mn[:], p[:])
                    M = Mn

            # ============ position diversity ============
            pa = per.tile([128, 1], f32, tag="pa")
            nc.scalar.activation(pa[:], cmb[:, POS_COL:POS_COL + 1], ACT.Abs)
            nc.vector.tensor_copy(slot(S_PASUM), psum_scalar(pa[:])[:])
            paT_p = psum([1, 128], "sc")
            nc.tensor.transpose(paT_p[:], pa[:], ident)
            paT = sml.tile([1, 128], f32, tag="paT")
            nc.vector.tensor_copy(paT[:], paT_p[:])
            nc.vector.tensor_reduce(slot(S_PAMAX), paT[:], axis=AX.X,
                                    op=ALU.max)

            # ============ herding MLP ============
            h1p = psum([128, 1], "sc")
            nc.tensor.matmul(h1p[:], lhsT=cmb[:, W1A_OFF:W1A_OFF + 128],
                             rhs=cmb[:, XL_COL:XL_COL + 1], start=True,
                             stop=False, skip_group_check=True)
            nc.tensor.matmul(h1p[:], lhsT=cmb[:, W1B_OFF:W1B_OFF + 128],
                             rhs=cmb[:, POS_COL:POS_COL + 1], start=False,
                             stop=True, skip_group_check=True)
            h1 = sml.tile([128, 1], f32, tag="h1")
            nc.scalar.activation(h1[:], h1p[:], ACT.Relu,
                                 bias=cmb[:, B1_COL:B1_COL + 1])
            gk = sml.tile([128, 1], f32, tag="gk")
            nc.vector.tensor_scalar(gk[:], cmb[:, GAM_COL:GAM_COL + 1],
                                    float(1.0 / np.sqrt(1.0 + 1e-5)), None,
                                    ALU.mult)
            h1b = sml.tile([128, 1], f32, tag="h1b")
            nc.vector.tensor_scalar(h1b[:], h1[:], gk[:],
                                    cmb[:, BET_COL:BET_COL + 1],
                                    ALU.mult, ALU.add)
            h2p = psum([64, 1], "sc")
            nc.tensor.matmul(h2p[:], lhsT=cmb[:, W2_OFF:W2_OFF + 64],
                             rhs=h1b[:], start=True, stop=True,
                             skip_group_check=True)
            h2 = sml.tile([64, 1], f32, tag="h2")
            nc.scalar.activation(h2[:], h2p[:], ACT.Relu,
                                 bias=cmb[0:64, B2_COL:B2_COL + 1])
            lg = psum([3, 1], "sc")
            nc.tensor.matmul(lg[:], lhsT=cmb[0:64, W3_OFF:W3_OFF + 3],
                             rhs=h2[:], start=True, stop=True,
                             skip_group_check=True)
            exps = sml.tile([3, 1], f32, tag="exps")
            nc.scalar.activation(exps[:], lg[:], ACT.Exp,
                                 bias=cmb[0:3, B3_COL:B3_COL + 1])
            esum = psum_scalar(exps[:], p=3)
            esum_sb = sml.tile([1, 1], f32, tag="esum_sb")
            nc.vector.tensor_copy(esum_sb[:], esum[:])
            erec = sml.tile([1, 1], f32, tag="erec")
            nc.vector.reciprocal(erec[:], esum_sb[:])
            e2p = psum([1, 1], "sc")
            nc.tensor.matmul(e2p[:], lhsT=cmb[0:3, OH2_COL:OH2_COL + 1],
                             rhs=exps[:], start=True, stop=True,
                             skip_group_check=True)
            e2_sb = sml.tile([1, 1], f32, tag="e2_sb")
            nc.vector.tensor_copy(e2_sb[:], e2p[:])
            nc.vector.tensor_mul(slot(S_SEV), e2_sb[:], erec[:])

            # ============ write out ============
            nc.sync.dma_start(out_d[:, :], out_sb[:])

    nc.compile()
    return nc


def _build_cmb_common(inputs):
    cmb = np.zeros((128, CMB_W), np.float32)
    for j in range(128):
        cmb[j, M20_OFF + j:M20_OFF + j + W20] = 1.0
        cmb[j, M10_OFF + j:M10_OFF + j + W10] = 1.0
    cmb[:, ID_OFF:ID_OFF + 128] = np.eye(128, dtype=np.float32)
    w1 = np.asarray(inputs["w1"], np.float32)
    cmb[:, W1A_OFF:W1A_OFF + 128] = w1[0:128]
    cmb[:, W1B_OFF:W1B_OFF + 128] = w1[128:256]
    cmb[:, W2_OFF:W2_OFF + 64] = np.asarray(inputs["w2"], np.float32)
    cmb[:, B1_COL] = np.asarray(inputs["b1"], np.float32)
    cmb[:, GAM_COL] = np.asarray(inputs["gamma"], np.float32)
    cmb[:, BET_COL] = np.asarray(inputs["beta"], np.float32)
    cmb[:, POS_COL] = np.asarray(inputs["positions"], np.float32)
    x = np.asarray(inputs["returns_sequence"], np.float32)
    cmb[:, XL_COL] = x[-1]
    cmb[127, OH127_COL] = 1.0
    cmb[0:64, W3_OFF:W3_OFF + 3] = np.asarray(inputs["w3"], np.float32)
    cmb[0:64, B2_COL] = np.asarray(inputs["b2"], np.float32)
    cmb[0:3, B3_COL] = np.asarray(inputs["b3"], np.float32)
    cmb[2, OH2_COL] = 1.0
    return cmb


def _prep_in_maps(inputs):
    x = np.ascontiguousarray(np.asarray(inputs["returns_sequence"],
                                        dtype=np.float32))
    cmb_common = _build_cmb_common(inputs)
    in_maps = []
    for c in range(NC_N):
        g = c * CHUNK + np.arange(CHUNK)
        cmb = cmb_common.copy()
        cmb[:, V20_OFF:V20_OFF + 8] = \
            (g < N20).astype(np.float32).reshape(8, 128).T
        cmb[:, H10_OFF:H10_OFF + 8] = \
            (g < N10 - 5).astype(np.float32).reshape(8, 128).T
        cmb[:, R10M_OFF:R10M_OFF + 8] = \
            ((g >= N10 - 5) & (g < N10)).astype(np.float32).reshape(8, 128).T
        rows = (c * CHUNK + np.arange(XROWS)) % T
        in_maps.append({
            "x_chunk": np.ascontiguousarray(
                x[c * CHUNK:(c + 1) * CHUNK]
                .reshape(8, 128, 128).transpose(1, 0, 2).reshape(128, CHUNK)),
            "xT_chunk": np.ascontiguousarray(x[rows].T),
            "cmb": np.ascontiguousarray(cmb),
        })
    return in_maps


def _combine(per_core):
    count20 = sum(float(per_core[c][0, S_COUNT20]) for c in range(NC_N))
    hist_y = sum(float(per_core[c][0, S_HIST10]) for c in range(NC_N))
    rec_y = sum(float(per_core[c][0, S_RECENT10]) for c in range(NC_N))
    cs_sum = sum(float(per_core[c][0, S_CSSUM]) for c in range(NC_N))
    ssq_sum = sum(float(per_core[c][0, S_SSQ]) for c in range(NC_N))
    cs_first = float(per_core[0][0, S_CSFIRST])
    cs_last = float(per_core[NC_N - 1][0, S_CSLAST])
    r0 = per_core[0][0]
    sum_corr = float(r0[S_SUMCORR])
    sum_abs = float(r0[S_SUMABS])
    trace_c = float(r0[S_TRACE])
    pa_sum = float(r0[S_PASUM])
    pa_max = float(r0[S_PAMAX])
    severity = float(r0[S_SEV])
    t7, t8 = float(r0[S_T7]), float(r0[S_T8])

    phase_locking = count20 / N20
    hist = (hist_y - A * (N10 - 5)) * INV_OD / (N10 - 5)
    recent = (rec_y - A * 5) * INV_OD / 5.0
    surge = 0.0
    if hist > 0:
        surge = min(max((recent - hist) / hist, 0.0), 1.0)
    avg_disp = cs_sum / T
    trend = -(cs_last - cs_first) / (T - 1)
    herding_index = min(max(trend / (avg_disp + 1e-6) + 0.5, 0.0), 1.0)
    avg_corr = (sum_corr - trace_c) * INV_OD
    lam = (t8 / t7) ** (1.0 / 128.0) if t7 > 0 and t8 > 0 else 1.0
    sync_risk = min(1.0, (lam / A) * avg_corr)
    return_div = 1.0 - sum_abs / (A * A)
    pos_div = 1.0 - pa_max / pa_sum
    div_loss = 1.0 - np.sqrt(return_div * pos_div)
    avg_conc = (A * A / 2.0 + ssq_sum / (2.0 * T) - A) / (A * (A - 1))
    phase_coupling = min(max((avg_conc - 0.5) * 2.0, 0.0), 1.0)
    collective = (herding_index + sync_risk + div_loss) / 3.0
    return np.array([herding_index, severity, sync_risk, phase_locking,
                     div_loss, surge, phase_coupling, collective],
                    dtype=np.float32)


def _ensure_ntff_hook():
    """Install the axon NTFF profile hook if the image lacks antenv.axon_hooks."""
    import sys
    import types
    try:
        import antenv.axon_hooks  # noqa: F401
        return True
    except ImportError:
        pass
    try:
        import antenv
        from trn_agent_boot.trn_boot import _ntff_profile_via_ctypes
        mod = types.ModuleType("antenv.axon_hooks")
        state = {}
        mod.set_axon_ntff_profile_hook = lambda h: state.update(h=h)
        mod.get_axon_ntff_profile_hook = lambda: state.get("h")
        sys.modules["antenv.axon_hooks"] = mod
        antenv.axon_hooks = mod
        hook = _ntff_profile_via_ctypes("/opt/axon/libaxon_pjrt.so")
        mod.set_axon_ntff_profile_hook(hook)
        return hook is not None
    except Exception:
        return False


def _run(inputs, trace=False):
    from concourse.bass_utils import run_bass_kernel_spmd
    if trace:
        trace = _ensure_ntff_hook()
    if "nc" not in _PLAN:
        _PLAN["nc"] = _build_program()
    nc = _PLAN["nc"]
    in_maps = _prep_in_maps(inputs)
    res = run_bass_kernel_spmd(nc, in_maps, core_ids=list(range(NC_N)),
                               trace=trace)
    per_core = [res.results[c]["out_vec"] for c in range(NC_N)]
    return _combine(per_core), res


def kernel(**inputs) -> np.ndarray:
    out, _ = _run(inputs, trace=False)
    return out


# revision 11
# speedup vs baseline: 1.6924x; 1.6924x over previous
"""Trainium2 Bass kernel for nn_EmergentRiskMetrics.

Contract: kernel(**inputs) takes the FULL unsharded inputs (as produced by
setup_inputs()) and returns the FULL output (shape [8], float32).

Sharding: data-parallel over the time axis. Each of the 8 cores owns 1024
contiguous rows (plus a window halo for the rolling scans). The full-T
[A,A] covariance is computed as per-core partials (8 bf16 matmuls on the
core's own rows) combined with a single 64KB AllReduce that overlaps the
rolling-window compute; the nonlinear [A,A] post-processing (correlation,
|corr| mean, top-eigenvalue power chain) and the tiny MLP are replicated.
The sign-concordance sum needs no matmul at all: sum_ab (S^T S)_ab =
sum_t (sum_a sign(x_ta))^2, which shards linearly over t.

Device outputs are per-core scalars; the host only gathers them (sums the
sharded partials, applies final scalar clips/divides) into the 8 outputs.
"""

import numpy as np

T = 8192
A = 128
W20 = 20
W10 = 10
NC_N = 8
CHUNK = 1024            # window starts / owned rows per core
XROWS = 1152            # rolling-scan rows incl. halo (9 x 128)
R20 = 128 + W20 - 1     # 147
R10 = 128 + W10 - 1     # 137
N20 = T - W20           # 8172 rolling-20 windows
N10 = T - W10           # 8182 rolling-10 windows
OUT_SLOTS = 16
INV_OD = 1.0 / (A * (A - 1))
# rolling > 0.7 in terms of the unnormalized window sum y:
# rolling = (y - A) * INV_OD  =>  y > 0.7 * A*(A-1) + A
Y_THRESH = 0.7 * A * (A - 1) + A

# out_vec slot layout
(S_COUNT20, S_HIST10, S_RECENT10, S_CSSUM, S_CSFIRST, S_CSLAST,
 S_SUMCORR, S_SUMABS, S_TRACE, S_PASUM, S_PAMAX, S_SEV, S_SSQ,
 S_T7, S_T8) = range(15)

# packed-constant column layout (cmb: [128, CMB_W] f32)
M20_OFF = 0
M10_OFF = M20_OFF + R20          # 147
V20_OFF = M10_OFF + R10          # 284
H10_OFF = V20_OFF + 8            # 292
R10M_OFF = H10_OFF + 8           # 300
ID_OFF = R10M_OFF + 8            # 308
W1A_OFF = ID_OFF + 128           # 436
W1B_OFF = W1A_OFF + 128          # 564
W2_OFF = W1B_OFF + 128           # 692
B1_COL = W2_OFF + 64             # 756
GAM_COL = B1_COL + 1             # 757
BET_COL = GAM_COL + 1            # 758
POS_COL = BET_COL + 1            # 759
XL_COL = POS_COL + 1             # 760
OH127_COL = XL_COL + 1           # 761
W3_OFF = OH127_COL + 1           # 762 (3 cols, rows 0:64)
B2_COL = W3_OFF + 3              # 765 (rows 0:64)
B3_COL = B2_COL + 1              # 766 (rows 0:3)
OH2_COL = B3_COL + 1             # 767 (rows 0:3)
CMB_W = OH2_COL + 1              # 768

_PLAN = {}


def _build_program():
    import concourse.bacc as bacc
    import concourse.tile as tile
    from concourse import mybir

    f32 = mybir.dt.float32
    bf16 = mybir.dt.bfloat16
    ALU = mybir.AluOpType
    ACT = mybir.ActivationFunctionType
    AX = mybir.AxisListType

    nc = bacc.Bacc("TRN2", target_bir_lowering=False, debug=False,
                   num_devices=NC_N)

    def din(name, shape):
        return nc.dram_tensor(name, shape, f32, kind="ExternalInput").ap()

    x_chunk = din("x_chunk", [128, CHUNK])     # row blocks: [p, j*128+a]
    xT_in = din("xT_chunk", [128, XROWS])      # [a, t_local]
    cmb_in = din("cmb", [128, CMB_W])
    xfb_in = nc.dram_tensor("x_full_b", [128, T], mybir.dt.bfloat16,
                            kind="ExternalInput").ap()
    out_d = nc.dram_tensor("out_vec", [1, OUT_SLOTS], f32,
                           kind="ExternalOutput").ap()

    with tile.TileContext(nc) as tc:
        with tc.tile_pool(name="const", bufs=1) as cst, \
             tc.tile_pool(name="persist", bufs=1) as per, \
             tc.tile_pool(name="work", bufs=3) as wrk, \
             tc.tile_pool(name="scr", bufs=4) as scp, \
             tc.tile_pool(name="small", bufs=6) as sml, \
             tc.tile_pool(name="dram", bufs=2, space="DRAM") as dram, \
             tc.tile_pool(name="ps", bufs=1, space="PSUM") as ps:

            psum_bufs = {"zp": 2, "big": 2, "sc": 2, "r2p": 2}

            def psum(shape, tag, dtype=f32):
                return ps.tile(shape, dtype, tag=tag, name=tag,
                               bufs=psum_bufs[tag])

            # ---- inputs ----
            xc = cst.tile([128, CHUNK], f32, tag="xc")
            nc.sync.dma_start(xc[:], x_chunk[:, :])
            xT = cst.tile([128, XROWS], f32, tag="xT")
            nc.sync.dma_start(xT[:], xT_in[:, :])
            cmb = cst.tile([128, CMB_W], f32, tag="cmb")
            nc.sync.dma_start(cmb[:], cmb_in[:, :])

            m20 = cmb[:, M20_OFF:M20_OFF + R20]
            m10 = cmb[:, M10_OFF:M10_OFF + R10]
            v20 = cmb[:, V20_OFF:V20_OFF + 8]
            h10 = cmb[:, H10_OFF:H10_OFF + 8]
            r10m = cmb[:, R10M_OFF:R10M_OFF + 8]
            ident = cmb[:, ID_OFF:ID_OFF + 128]

            ones = cst.tile([128, 1], f32, tag="ones")
            nc.vector.memset(ones[:], 1.0)
            onesb = cst.tile([128, 1], bf16, tag="onesb")
            nc.vector.memset(onesb[:], 1.0)
            mb20 = cst.tile([128, R20], bf16, tag="mb20")
            nc.scalar.activation(mb20[:], cmb[:, M20_OFF:M20_OFF + R20],
                                 ACT.Copy)
            mb10 = cst.tile([128, R10], bf16, tag="mb10")
            nc.scalar.activation(mb10[:], cmb[:, M10_OFF:M10_OFF + R10],
                                 ACT.Copy)

            out_sb = per.tile([1, OUT_SLOTS], f32, tag="out_sb")
            nc.vector.memset(out_sb[:], 0.0)

            def slot(i):
                return out_sb[:, i:i + 1]

            # sum over partitions of an SBUF [p,1] vector -> [1,1] psum
            def psum_scalar(vec_sb, p=128):
                o = psum([1, 1], "sc")
                lhs = ones[0:p, :] if p != 128 else ones[:]
                nc.tensor.matmul(o[:], lhsT=lhs, rhs=vec_sb,
                                 start=True, stop=True, skip_group_check=True)
                return o

            # ============ Phase A: full-T covariance (replicated, bf16) ======
            # x_full_b is the host-cast bf16 copy of x in [p, i*128+a] layout
            xfbs = []
            for j in range(8):
                xfb = cst.tile([128, CHUNK], bf16, tag="xfb%d" % j)
                nc.sync.dma_start(xfb[:], xfb_in[:, j * CHUNK:(j + 1) * CHUNK])
                xfbs.append(xfb)
            covp = psum([128, 128], "big")
            for i in range(64):
                blk = xfbs[i // 8][:, (i % 8) * 128:(i % 8 + 1) * 128]
                nc.tensor.matmul(covp[:], lhsT=blk, rhs=blk,
                                 start=(i == 0), stop=(i == 63),
                                 skip_group_check=True)
            cov = per.tile([128, 128], f32, tag="cov")
            nc.vector.tensor_copy(cov[:], covp[:])
            # per-t cross-asset sums for the cross-sectional std
            sr = per.tile([128, 8], f32, tag="sr")
            for j in range(8):
                scr = scp.tile([128, 128], bf16, tag="scrB")
                nc.scalar.activation(scr[:], xc[:, j * 128:(j + 1) * 128],
                                     ACT.Copy, accum_out=sr[:, j:j + 1])

            # ============ Phase B: row stats (csstd, sign concordance) =======
            ss = per.tile([128, 8], f32, tag="ss")
            rho = per.tile([128, 8], f32, tag="rho")
            for j in range(8):
                scr = scp.tile([128, 128], bf16, tag="scrB")
                nc.scalar.activation(scr[:], xc[:, j * 128:(j + 1) * 128],
                                     ACT.Square, accum_out=ss[:, j:j + 1])
            for j in range(8):
                scr = scp.tile([128, 128], bf16, tag="scrB")
                nc.scalar.activation(scr[:], xc[:, j * 128:(j + 1) * 128],
                                     ACT.Sign, accum_out=rho[:, j:j + 1])

            # cross-sectional std per t: sqrt((ss - sr^2/A) / (A-1))
            sq2 = sml.tile([128, 8], f32, tag="sq2")
            nc.scalar.activation(sq2[:], sr[:], ACT.Square,
                                 scale=float(1.0 / np.sqrt(A)))
            varA = sml.tile([128, 8], f32, tag="varA")
            nc.vector.scalar_tensor_tensor(
                varA[:], in0=sq2[:], scalar=-1.0, in1=ss[:],
                op0=ALU.mult, op1=ALU.add)
            csstd = per.tile([128, 8], f32, tag="csstd")
            nc.scalar.activation(csstd[:], varA[:], ACT.Sqrt,
                                 scale=float(1.0 / (A - 1)))
            csr = sml.tile([128, 1], f32, tag="csr")
            nc.vector.tensor_reduce(csr[:], csstd[:], axis=AX.X, op=ALU.add)
            nc.vector.tensor_copy(slot(S_CSSUM), psum_scalar(csr[:])[:])
            nc.vector.tensor_copy(slot(S_CSFIRST), csstd[0:1, 0:1])
            cslast_p = psum([1, 1], "sc")
            nc.tensor.matmul(cslast_p[:], lhsT=cmb[:, OH127_COL:OH127_COL + 1],
                             rhs=csstd[:, 7:8],
                             start=True, stop=True, skip_group_check=True)
            nc.vector.tensor_copy(slot(S_CSLAST), cslast_p[:])

            # ssq partial: sum_t rho_t^2
            rr = sml.tile([128, 1], f32, tag="rr")
            rho2 = sml.tile([128, 8], f32, tag="rho2")
            nc.scalar.activation(rho2[:], rho[:], ACT.Square, accum_out=rr[:])
            nc.vector.tensor_copy(slot(S_SSQ), psum_scalar(rr[:])[:])

            # ============ Phase C: rolling windows ============
            xTb = per.tile([128, XROWS], bf16, tag="xTb")
            nc.scalar.activation(xTb[:], xT[:], ACT.Copy)
            x2T = per.tile([128, 1151], bf16, tag="x2T")
            nc.scalar.activation(x2T[:], xT[:, 0:1151], ACT.Square)

            # rolling sums via log-shift adds (S on vector, P=sum x^2 on gpsimd)
            def rollsums(src, tag, eng, w0):
                s2 = per.tile([128, w0], bf16, tag=tag + "s2")
                eng.tensor_add(s2[:], src[:, 0:w0], src[:, 1:w0 + 1])
                s4 = per.tile([128, w0 - 2], bf16, tag=tag + "s4")
                eng.tensor_add(s4[:], s2[:, 0:w0 - 2], s2[:, 2:w0])
                s8 = per.tile([128, w0 - 6], bf16, tag=tag + "s8")
                eng.tensor_add(s8[:], s4[:, 0:w0 - 6], s4[:, 4:w0 - 2])
                s16 = per.tile([128, w0 - 14], bf16, tag=tag + "s16")
                eng.tensor_add(s16[:], s8[:, 0:w0 - 14], s8[:, 8:w0 - 6])
                s20 = per.tile([128, CHUNK], bf16, tag=tag + "s20")
                eng.tensor_add(s20[:], s16[:, 0:CHUNK], s4[:, 16:CHUNK + 16])
                s10 = per.tile([128, CHUNK], bf16, tag=tag + "s10")
                eng.tensor_add(s10[:], s8[:, 0:CHUNK], s2[:, 8:CHUNK + 8])
                return s20, s10

            S20, S10 = rollsums(xTb, "S", nc.vector, 1151)
            P20, P10 = rollsums(x2T, "P", nc.gpsimd, 1150)

            # u = 1/sqrt(P - S^2/w), bf16 for the z-matmul lhsT
            def make_u(S, P, w, tag):
                ssq_ = per.tile([128, CHUNK], bf16, tag=tag + "ssq")
                nc.scalar.activation(ssq_[:], S[:], ACT.Square,
                                     scale=float(1.0 / np.sqrt(w)))
                d2 = per.tile([128, CHUNK], f32, tag=tag + "d2")
                nc.gpsimd.tensor_tensor(d2[:], P[:], ssq_[:], ALU.subtract)
                rcp = per.tile([128, CHUNK], f32, tag=tag + "rcp")
                nc.vector.reciprocal_approx_fast(rcp[:], d2[:])
                u = per.tile([128, CHUNK], bf16, tag=tag + "u")
                nc.scalar.activation(u[:], rcp[:], ACT.Sqrt)
                return u

            u20 = make_u(S20, P20, W20, "u20")
            u10 = make_u(S10, P10, W10, "u10")
            us20 = per.tile([128, CHUNK], bf16, tag="us20")
            nc.gpsimd.tensor_mul(us20[:], u20[:], S20[:])
            us10 = per.tile([128, CHUNK], bf16, tag="us10")
            nc.gpsimd.tensor_mul(us10[:], u10[:], S10[:])

            # per-chunk window sums y = r1 - r2^2/w  (rolling = (y-A)*inv_od)
            y20 = per.tile([128, 8], f32, tag="y20")
            y10 = per.tile([128, 8], f32, tag="y10")

            def roll_chunk(k, u, us, R, mb, w, ycol):
                ksl = slice(k * 128, (k + 1) * 128)
                zp = psum([128, R20], "zp")
                nc.tensor.matmul(zp[:, 0:R], lhsT=u[:, ksl],
                                 rhs=xTb[:, k * 128:k * 128 + R],
                                 start=True, stop=True, skip_group_check=True)
                r2p = psum([128, 1], "r2p")
                nc.tensor.matmul(r2p[:], lhsT=us[:, ksl], rhs=onesb[:],
                                 start=True, stop=True, skip_group_check=True)
                V = wrk.tile([128, R20], bf16, tag="V")
                nc.scalar.activation(V[:, 0:R], zp[:, 0:R], ACT.Square)
                rsqw = sml.tile([128, 1], f32, tag="rsqw")
                nc.scalar.activation(rsqw[:], r2p[:], ACT.Square,
                                     scale=float(1.0 / np.sqrt(w)))
                Vm = wrk.tile([128, R20], bf16, tag="Vm")
                nc.vector.tensor_mul(Vm[:, 0:R], V[:, 0:R], mb)
                r1 = sml.tile([128, 1], f32, tag="r1")
                nc.vector.tensor_reduce(r1[:], Vm[:, 0:R], axis=AX.X,
                                        op=ALU.add)
                nc.vector.scalar_tensor_tensor(
                    ycol, in0=rsqw[:], scalar=-1.0, in1=r1[:],
                    op0=ALU.mult, op1=ALU.add)

            for k in range(8):
                roll_chunk(k, u20, us20, R20, mb20, W20, y20[:, k:k + 1])
                roll_chunk(k, u10, us10, R10, mb10, W10, y10[:, k:k + 1])

            # batched finals
            c1 = sml.tile([128, 8], f32, tag="c1")
            nc.vector.tensor_scalar(c1[:], y20[:], float(Y_THRESH), None,
                                    ALU.is_gt)
            c2 = sml.tile([128, 8], f32, tag="c2")
            nc.vector.tensor_mul(c2[:], c1[:], v20)
            cnt = sml.tile([128, 1], f32, tag="cnt")
            nc.vector.tensor_reduce(cnt[:], c2[:], axis=AX.X, op=ALU.add)
            nc.vector.tensor_copy(slot(S_COUNT20), psum_scalar(cnt[:])[:])
            hscr = sml.tile([128, 8], f32, tag="hscr")
            nc.vector.tensor_mul(hscr[:], y10[:], h10)
            hs = sml.tile([128, 1], f32, tag="hs")
            nc.vector.tensor_reduce(hs[:], hscr[:], axis=AX.X, op=ALU.add)
            nc.vector.tensor_copy(slot(S_HIST10), psum_scalar(hs[:])[:])
            rscr = sml.tile([128, 8], f32, tag="rscr")
            nc.vector.tensor_mul(rscr[:], y10[:], r10m)
            rs = sml.tile([128, 1], f32, tag="rs")
            nc.vector.tensor_reduce(rs[:], rscr[:], axis=AX.X, op=ALU.add)
            nc.vector.tensor_copy(slot(S_RECENT10), psum_scalar(rs[:])[:])

            # ============ Phase D: [A,A] post-processing ============
            dscr = wrk.tile([128, 128], f32, tag="dscr")
            nc.vector.tensor_mul(dscr[:], cov[:], ident)
            diag = sml.tile([128, 1], f32, tag="diag")
            nc.vector.tensor_reduce(diag[:], dscr[:], axis=AX.X, op=ALU.add)
            dstd = sml.tile([128, 1], f32, tag="dstd")
            nc.scalar.activation(dstd[:], diag[:], ACT.Sqrt)
            ucol = per.tile([128, 1], f32, tag="ucol")
            nc.vector.reciprocal(ucol[:], dstd[:])
            u2 = sml.tile([128, 1], f32, tag="u2")
            nc.vector.tensor_mul(u2[:], ucol[:], ucol[:])
            du2 = sml.tile([128, 1], f32, tag="du2")
            nc.vector.tensor_mul(du2[:], u2[:], diag[:])
            nc.vector.tensor_copy(slot(S_TRACE), psum_scalar(du2[:])[:])

            uT_p = psum([1, 128], "sc")
            nc.tensor.transpose(uT_p[:], ucol[:], ident)
            uT = per.tile([1, 128], f32, tag="uT")
            nc.vector.tensor_copy(uT[:], uT_p[:])

            def quad_form(mat_sb, out_slot):
                qr = psum([1, 128], "sc")
                nc.tensor.matmul(qr[:], lhsT=ucol[:], rhs=mat_sb,
                                 start=True, stop=True, skip_group_check=True)
                qscr = sml.tile([1, 128], f32, tag="qscr")
                nc.vector.tensor_mul(qscr[:], qr[:], uT[:])
                qacc = sml.tile([1, 1], f32, tag="qacc")
                nc.vector.tensor_reduce(qacc[:], qscr[:], axis=AX.X,
                                        op=ALU.add)
                nc.vector.tensor_copy(out_slot, qacc[:])

            quad_form(cov[:], slot(S_SUMCORR))
            acov = wrk.tile([128, 128], f32, tag="acov")
            nc.scalar.activation(acov[:], cov[:], ACT.Abs)
            quad_form(acov[:], slot(S_SUMABS))

            # corr in bf16: diag(u) cov diag(u) via row-scale, transpose, scale
            brow = wrk.tile([128, 128], f32, tag="brow")
            nc.vector.tensor_scalar(brow[:], cov[:], ucol[:], None, ALU.mult)
            bt_p = psum([128, 128], "big")
            nc.tensor.transpose(bt_p[:], brow[:], ident)
            corrb = per.tile([128, 128], bf16, tag="corrb")
            nc.vector.tensor_scalar(corrb[:], bt_p[:], ucol[:], None, ALU.mult)

            # top eigenvalue: 8 bf16 squarings; traces of M^128 and M^256
            M = corrb
            for kk in range(8):
                p = psum([128, 128], "big")
                nc.tensor.matmul(p[:], lhsT=M[:], rhs=M[:],
                                 start=True, stop=True, skip_group_check=True)
                if kk >= 6:
                    escr = wrk.tile([128, 128], f32, tag="escr")
                    nc.vector.tensor_mul(escr[:], p[:], ident)
                    edg = sml.tile([128, 1], f32, tag="edg")
                    nc.vector.tensor_reduce(edg[:], escr[:], axis=AX.X,
                                            op=ALU.add)
                    s = S_T7 if kk == 6 else S_T8
                    nc.vector.tensor_copy(slot(s), psum_scalar(edg[:])[:])
                if kk < 7:
                    Mn = wrk.tile([128, 128], bf16, tag="Mn", bufs=2)
                    nc.vector.tensor_copy(Mn[:], p[:])
                    M = Mn

            # ============ position diversity ============
            pa = per.tile([128, 1], f32, tag="pa")
            nc.scalar.activation(pa[:], cmb[:, POS_COL:POS_COL + 1], ACT.Abs)
            nc.vector.tensor_copy(slot(S_PASUM), psum_scalar(pa[:])[:])
            paT_p = psum([1, 128], "sc")
            nc.tensor.transpose(paT_p[:], pa[:], ident)
            paT = sml.tile([1, 128], f32, tag="paT")
            nc.vector.tensor_copy(paT[:], paT_p[:])
            nc.vector.tensor_reduce(slot(S_PAMAX), paT[:], axis=AX.X,
                                    op=ALU.max)

            # ============ herding MLP ============
            h1p = psum([128, 1], "sc")
            nc.tensor.matmul(h1p[:], lhsT=cmb[:, W1A_OFF:W1A_OFF + 128],
                             rhs=cmb[:, XL_COL:XL_COL + 1], start=True,
                             stop=False, skip_group_check=True)
            nc.tensor.matmul(h1p[:], lhsT=cmb[:, W1B_OFF:W1B_OFF + 128],
                             rhs=cmb[:, POS_COL:POS_COL + 1], start=False,
                             stop=True, skip_group_check=True)
            h1 = sml.tile([128, 1], f32, tag="h1")
            nc.scalar.activation(h1[:], h1p[:], ACT.Relu,
                                 bias=cmb[:, B1_COL:B1_COL + 1])
            gk = sml.tile([128, 1], f32, tag="gk")
            nc.vector.tensor_scalar(gk[:], cmb[:, GAM_COL:GAM_COL + 1],
                                    float(1.0 / np.sqrt(1.0 + 1e-5)), None,
                                    ALU.mult)
            h1b = sml.tile([128, 1], f32, tag="h1b")
            nc.vector.tensor_scalar(h1b[:], h1[:], gk[:],
                                    cmb[:, BET_COL:BET_COL + 1],
                                    ALU.mult, ALU.add)
            h2p = psum([64, 1], "sc")
            nc.tensor.matmul(h2p[:], lhsT=cmb[:, W2_OFF:W2_OFF + 64],
                             rhs=h1b[:], start=True, stop=True,
                             skip_group_check=True)
            h2 = sml.tile([64, 1], f32, tag="h2")
            nc.scalar.activation(h2[:], h2p[:], ACT.Relu,
                                 bias=cmb[0:64, B2_COL:B2_COL + 1])
            lg = psum([3, 1], "sc")
            nc.tensor.matmul(lg[:], lhsT=cmb[0:64, W3_OFF:W3_OFF + 3],
                             rhs=h2[:], start=True, stop=True,
                             skip_group_check=True)
            exps = sml.tile([3, 1], f32, tag="exps")
            nc.scalar.activation(exps[:], lg[:], ACT.Exp,
                                 bias=cmb[0:3, B3_COL:B3_COL + 1])
            esum = psum_scalar(exps[:], p=3)
            esum_sb = sml.tile([1, 1], f32, tag="esum_sb")
            nc.vector.tensor_copy(esum_sb[:], esum[:])
            erec = sml.tile([1, 1], f32, tag="erec")
            nc.vector.reciprocal(erec[:], esum_sb[:])
            e2p = psum([1, 1], "sc")
            nc.tensor.matmul(e2p[:], lhsT=cmb[0:3, OH2_COL:OH2_COL + 1],
                             rhs=exps[:], start=True, stop=True,
                             skip_group_check=True)
            e2_sb = sml.tile([1, 1], f32, tag="e2_sb")
            nc.vector.tensor_copy(e2_sb[:], e2p[:])
            nc.vector.tensor_mul(slot(S_SEV), e2_sb[:], erec[:])

            # ============ write out ============
            nc.sync.dma_start(out_d[:, :], out_sb[:])

    nc.compile()
    return nc


def _build_cmb_common(inputs):
    cmb = np.zeros((128, CMB_W), np.float32)
    for j in range(128):
        cmb[j, M20_OFF + j:M20_OFF + j + W20] = 1.0
        cmb[j, M10_OFF + j:M10_OFF + j + W10] = 1.0
    cmb[:, ID_OFF:ID_OFF + 128] = np.eye(128, dtype=np.float32)
    w1 = np.asarray(inputs["w1"], np.float32)
    cmb[:, W1A_OFF:W1A_OFF + 128] = w1[0:128]
    cmb[:, W1B_OFF:W1B_OFF + 128] = w1[128:256]
    cmb[:, W2_OFF:W2_OFF + 64] = np.asarray(inputs["w2"], np.float32)
    cmb[:, B1_COL] = np.asarray(inputs["b1"], np.float32)
    cmb[:, GAM_COL] = np.asarray(inputs["gamma"], np.float32)
    cmb[:, BET_COL] = np.asarray(inputs["beta"], np.float32)
    cmb[:, POS_COL] = np.asarray(inputs["positions"], np.float32)
    x = np.asarray(inputs["returns_sequence"], np.float32)
    cmb[:, XL_COL] = x[-1]
    cmb[127, OH127_COL] = 1.0
    cmb[0:64, W3_OFF:W3_OFF + 3] = np.asarray(inputs["w3"], np.float32)
    cmb[0:64, B2_COL] = np.asarray(inputs["b2"], np.float32)
    cmb[0:3, B3_COL] = np.asarray(inputs["b3"], np.float32)
    cmb[2, OH2_COL] = 1.0
    return cmb


def _prep_in_maps(inputs):
    import ml_dtypes
    x = np.ascontiguousarray(np.asarray(inputs["returns_sequence"],
                                        dtype=np.float32))
    xfb = np.ascontiguousarray(
        x.reshape(64, 128, 128).transpose(1, 0, 2).reshape(128, T)
        .astype(ml_dtypes.bfloat16))
    cmb_common = _build_cmb_common(inputs)
    in_maps = []
    for c in range(NC_N):
        g = c * CHUNK + np.arange(CHUNK)
        cmb = cmb_common.copy()
        cmb[:, V20_OFF:V20_OFF + 8] = \
            (g < N20).astype(np.float32).reshape(8, 128).T
        cmb[:, H10_OFF:H10_OFF + 8] = \
            (g < N10 - 5).astype(np.float32).reshape(8, 128).T
        cmb[:, R10M_OFF:R10M_OFF + 8] = \
            ((g >= N10 - 5) & (g < N10)).astype(np.float32).reshape(8, 128).T
        rows = (c * CHUNK + np.arange(XROWS)) % T
        in_maps.append({
            "x_chunk": np.ascontiguousarray(
                x[c * CHUNK:(c + 1) * CHUNK]
                .reshape(8, 128, 128).transpose(1, 0, 2).reshape(128, CHUNK)),
            "xT_chunk": np.ascontiguousarray(x[rows].T),
            "cmb": np.ascontiguousarray(cmb),
            "x_full_b": xfb,
        })
    return in_maps


def _combine(per_core):
    count20 = sum(float(per_core[c][0, S_COUNT20]) for c in range(NC_N))
    hist_y = sum(float(per_core[c][0, S_HIST10]) for c in range(NC_N))
    rec_y = sum(float(per_core[c][0, S_RECENT10]) for c in range(NC_N))
    cs_sum = sum(float(per_core[c][0, S_CSSUM]) for c in range(NC_N))
    ssq_sum = sum(float(per_core[c][0, S_SSQ]) for c in range(NC_N))
    cs_first = float(per_core[0][0, S_CSFIRST])
    cs_last = float(per_core[NC_N - 1][0, S_CSLAST])
    r0 = per_core[0][0]
    sum_corr = float(r0[S_SUMCORR])
    sum_abs = float(r0[S_SUMABS])
    trace_c = float(r0[S_TRACE])
    pa_sum = float(r0[S_PASUM])
    pa_max = float(r0[S_PAMAX])
    severity = float(r0[S_SEV])
    t7, t8 = float(r0[S_T7]), float(r0[S_T8])

    phase_locking = count20 / N20
    hist = (hist_y - A * (N10 - 5)) * INV_OD / (N10 - 5)
    recent = (rec_y - A * 5) * INV_OD / 5.0
    surge = 0.0
    if hist > 0:
        surge = min(max((recent - hist) / hist, 0.0), 1.0)
    avg_disp = cs_sum / T
    trend = -(cs_last - cs_first) / (T - 1)
    herding_index = min(max(trend / (avg_disp + 1e-6) + 0.5, 0.0), 1.0)
    avg_corr = (sum_corr - trace_c) * INV_OD
    lam = (t8 / t7) ** (1.0 / 128.0) if t7 > 0 and t8 > 0 else 1.0
    sync_risk = min(1.0, (lam / A) * avg_corr)
    return_div = 1.0 - sum_abs / (A * A)
    pos_div = 1.0 - pa_max / pa_sum
    div_loss = 1.0 - np.sqrt(return_div * pos_div)
    avg_conc = (A * A / 2.0 + ssq_sum / (2.0 * T) - A) / (A * (A - 1))
    phase_coupling = min(max((avg_conc - 0.5) * 2.0, 0.0), 1.0)
    collective = (herding_index + sync_risk + div_loss) / 3.0
    return np.array([herding_index, severity, sync_risk, phase_locking,
                     div_loss, surge, phase_coupling, collective],
                    dtype=np.float32)


def _ensure_ntff_hook():
    """Install the axon NTFF profile hook if the image lacks antenv.axon_hooks."""
    import sys
    import types
    try:
        import antenv.axon_hooks  # noqa: F401
        return True
    except ImportError:
        pass
    try:
        import antenv
        from trn_agent_boot.trn_boot import _ntff_profile_via_ctypes
        mod = types.ModuleType("antenv.axon_hooks")
        state = {}
        mod.set_axon_ntff_profile_hook = lambda h: state.update(h=h)
        mod.get_axon_ntff_profile_hook = lambda: state.get("h")
        sys.modules["antenv.axon_hooks"] = mod
        antenv.axon_hooks = mod
        hook = _ntff_profile_via_ctypes("/opt/axon/libaxon_pjrt.so")
        mod.set_axon_ntff_profile_hook(hook)
        return hook is not None
    except Exception:
        return False


def _run(inputs, trace=False):
    from concourse.bass_utils import run_bass_kernel_spmd
    if trace:
        trace = _ensure_ntff_hook()
    if "nc" not in _PLAN:
        _PLAN["nc"] = _build_program()
    nc = _PLAN["nc"]
    in_maps = _prep_in_maps(inputs)
    res = run_bass_kernel_spmd(nc, in_maps, core_ids=list(range(NC_N)),
                               trace=trace)
    per_core = [res.results[c]["out_vec"] for c in range(NC_N)]
    return _combine(per_core), res


def kernel(**inputs) -> np.ndarray:
    out, _ = _run(inputs, trace=False)
    return out


# revision 14
# speedup vs baseline: 1.8564x; 1.0969x over previous
"""Trainium2 Bass kernel for nn_EmergentRiskMetrics.

Contract: kernel(**inputs) takes the FULL unsharded inputs (as produced by
setup_inputs()) and returns the FULL output (shape [8], float32).

Sharding: data-parallel over the time axis. Each of the 8 cores owns 1024
contiguous rolling-window starts (plus halo) and 1024 rows of the
cross-sectional / sign-concordance statistics; those shard linearly and the
host sums the per-core scalars. The [A,A] covariance work (bf16 matmul
accumulation over all T rows), its nonlinear post-processing (correlation,
|corr| mean, top-eigenvalue power chain) and the tiny MLP are replicated on
every core - a cross-core AllReduce was measured at ~94us fixed cost in
this runtime, far more than recomputing the 64 bf16 matmuls locally.

Engine budget drives the layout: fp32 for all elementwise work on the
Vector/GpSimd engines (bf16 elementwise is ~3x slower there), bf16 only as
matmul operands; row-layout statistics come from ones-matmuls on the
otherwise idle tensor engine; the w=10 rolling pipeline is scheduled before
w=20 so its normalizers are ready the moment the covariance matmuls drain.

Device outputs are per-core scalars; the host only gathers them (sums the
sharded partials, applies final scalar clips/divides) into the 8 outputs.
"""

import numpy as np

T = 8192
A = 128
W20 = 20
W10 = 10
NC_N = 8
CHUNK = 1024            # window starts / owned rows per core
XROWS = 1152            # rolling-scan rows incl. halo (9 x 128)
RW = 160                # z-matmul columns per window chunk (>= 128+w-1)
N20 = T - W20           # 8172 rolling-20 windows
N10 = T - W10           # 8182 rolling-10 windows
OUT_SLOTS = 16
INV_OD = 1.0 / (A * (A - 1))
# rolling > 0.7 in terms of the unnormalized window sum y:
# rolling = (y - A) * INV_OD  =>  y > 0.7 * A*(A-1) + A
Y_THRESH = 0.7 * A * (A - 1) + A

# out_vec slot layout
(S_COUNT20, S_HIST10, S_RECENT10, S_CSSUM, S_CSFIRST, S_CSLAST,
 S_SUMCORR, S_SUMABS, S_TRACE, S_PASUM, S_PAMAX, S_SEV, S_SSQ,
 S_T6, S_T7) = range(15)

# packed-constant column layout (cmb: [128, CMB_W] f32)
MCAT_OFF = 0                      # [128, 2*RW]: w20 mask | w10 mask
THRV_OFF = MCAT_OFF + 2 * RW      # 320  [128,16] interleaved thresholds
V20X_OFF = THRV_OFF + 16          # 336  valid20 on even cols, 0 on odd
H10X_OFF = V20X_OFF + 16          # 352  hist10 on odd cols, 0 on even
R10X_OFF = H10X_OFF + 16          # 368  recent10 on odd cols, 0 on even
ID_OFF = R10X_OFF + 16            # 384
W1A_OFF = ID_OFF + 128            # 512
W1B_OFF = W1A_OFF + 128           # 640
W2_OFF = W1B_OFF + 128            # 768
B1_COL = W2_OFF + 64              # 832
GAM_COL = B1_COL + 1              # 833
BET_COL = GAM_COL + 1             # 834
POS_COL = BET_COL + 1             # 835
XL_COL = POS_COL + 1              # 836
OH127_COL = XL_COL + 1            # 837
W3_OFF = OH127_COL + 1            # 838 (3 cols, rows 0:64)
B2_COL = W3_OFF + 3               # 841 (rows 0:64)
B3_COL = B2_COL + 1               # 842 (rows 0:3)
OH2_COL = B3_COL + 1              # 843 (rows 0:3)
CMB_W = OH2_COL + 1               # 844

_PLAN = {}


def _build_program():
    import concourse.bacc as bacc
    import concourse.tile as tile
    from concourse import mybir

    f32 = mybir.dt.float32
    bf16 = mybir.dt.bfloat16
    ALU = mybir.AluOpType
    ACT = mybir.ActivationFunctionType
    AX = mybir.AxisListType

    nc = bacc.Bacc("TRN2", target_bir_lowering=False, debug=False,
                   num_devices=NC_N)

    xfb_in = nc.dram_tensor("x_full_b", [128, T], bf16,
                            kind="ExternalInput").ap()
    xT_in = nc.dram_tensor("xT_chunk", [128, XROWS], f32,
                           kind="ExternalInput").ap()
    xTb_in = nc.dram_tensor("xTb_chunk", [128, XROWS], bf16,
                            kind="ExternalInput").ap()
    cmb_in = nc.dram_tensor("cmb", [128, CMB_W], f32,
                            kind="ExternalInput").ap()
    out_d = nc.dram_tensor("out_vec", [1, OUT_SLOTS], f32,
                           kind="ExternalOutput").ap()

    with tile.TileContext(nc) as tc:
        with tc.tile_pool(name="const", bufs=1) as cst, \
             tc.tile_pool(name="persist", bufs=1) as per, \
             tc.tile_pool(name="work", bufs=3) as wrk, \
             tc.tile_pool(name="small", bufs=6) as sml, \
             tc.tile_pool(name="ps", bufs=1, space="PSUM") as ps:

            psum_bufs = {"zp": 2, "big": 2, "sc": 1, "r2p": 1, "stat": 1}

            def psum(shape, tag):
                return ps.tile(shape, f32, tag=tag, name=tag,
                               bufs=psum_bufs[tag])

            # ---- input DMAs, spread across engine queues ----
            xT = cst.tile([128, XROWS], f32, tag="xT")
            nc.sync.dma_start(xT[:], xT_in[:, :])
            xfbs = [cst.tile([128, CHUNK], bf16, tag="xfb%d" % j,
                             name="xfb%d" % j)
                    for j in range(8)]
            nc.sync.dma_start(xfbs[0][:], xfb_in[:, 0:CHUNK])
            nc.sync.dma_start(xfbs[1][:], xfb_in[:, CHUNK:2 * CHUNK])
            xTb = cst.tile([128, XROWS], bf16, tag="xTb")
            nc.sync.dma_start(xTb[:], xTb_in[:, :])
            nc.sync.dma_start(xfbs[2][:], xfb_in[:, 2 * CHUNK:3 * CHUNK])
            nc.sync.dma_start(xfbs[3][:], xfb_in[:, 3 * CHUNK:4 * CHUNK])
            for j in range(4, 8):
                nc.gpsimd.dma_start(xfbs[j][:],
                                    xfb_in[:, j * CHUNK:(j + 1) * CHUNK])
            cmb = cst.tile([128, CMB_W], f32, tag="cmb")
            nc.gpsimd.dma_start(cmb[:], cmb_in[:, :])

            mcat = cmb[:, MCAT_OFF:MCAT_OFF + 2 * RW]
            thrv = cmb[:, THRV_OFF:THRV_OFF + 16]
            v20x = cmb[:, V20X_OFF:V20X_OFF + 16]
            h10x = cmb[:, H10X_OFF:H10X_OFF + 16]
            r10x = cmb[:, R10X_OFF:R10X_OFF + 16]
            ident = cmb[:, ID_OFF:ID_OFF + 128]

            ones = cst.tile([128, 1], f32, tag="ones")
            nc.vector.memset(ones[:], 1.0)
            onesb = cst.tile([128, 1], bf16, tag="onesb")
            nc.vector.memset(onesb[:], 1.0)
            ow20 = cst.tile([128, 1], bf16, tag="ow20")
            nc.vector.memset(ow20[:], float(1.0 / np.sqrt(W20)))
            ow10 = cst.tile([128, 1], bf16, tag="ow10")
            nc.vector.memset(ow10[:], float(1.0 / np.sqrt(W10)))

            out_sb = per.tile([1, OUT_SLOTS], f32, tag="out_sb")
            nc.vector.memset(out_sb[:], 0.0)

            def slot(i):
                return out_sb[:, i:i + 1]

            def psum_scalar(vec_sb, p=128):
                o = psum([1, 1], "sc")
                lhs = ones[0:p, :] if p != 128 else ones[:]
                nc.tensor.matmul(o[:], lhsT=lhs, rhs=vec_sb,
                                 start=True, stop=True, skip_group_check=True)
                return o

            # ============ scalar: squares / signs feeding the chains =========
            x2T = per.tile([128, 1151], f32, tag="x2T")
            nc.scalar.activation(x2T[:], xT[:, 0:1151], ACT.Square)
            sgT = per.tile([128, CHUNK], bf16, tag="sgT")
            nc.scalar.activation(sgT[:], xTb[:, 0:CHUNK], ACT.Sign)

            # ============ rolling sums (S on vector, P=sum x^2 on gpsimd) ====
            # w=10 parts first so its normalizer is ready when cov drains
            s2 = per.tile([128, 1151], f32, tag="Ss2")
            nc.vector.tensor_add(s2[:], xT[:, 0:1151], xT[:, 1:1152])
            s4 = per.tile([128, 1149], f32, tag="Ss4")
            nc.vector.tensor_add(s4[:], s2[:, 0:1149], s2[:, 2:1151])
            s8 = per.tile([128, 1145], f32, tag="Ss8")
            nc.vector.tensor_add(s8[:], s4[:, 0:1145], s4[:, 4:1149])
            S10 = per.tile([128, CHUNK], f32, tag="S10")
            nc.vector.tensor_add(S10[:], s8[:, 0:CHUNK], s2[:, 8:CHUNK + 8])
            s16 = per.tile([128, 1137], f32, tag="Ss16")
            nc.vector.tensor_add(s16[:], s8[:, 0:1137], s8[:, 8:1145])
            S20 = per.tile([128, CHUNK], f32, tag="S20")
            nc.vector.tensor_add(S20[:], s16[:, 0:CHUNK], s4[:, 16:CHUNK + 16])

            p2 = per.tile([128, 1150], f32, tag="Pp2")
            nc.gpsimd.tensor_add(p2[:], x2T[:, 0:1150], x2T[:, 1:1151])
            p4 = per.tile([128, 1148], f32, tag="Pp4")
            nc.gpsimd.tensor_add(p4[:], p2[:, 0:1148], p2[:, 2:1150])
            p8 = per.tile([128, 1144], f32, tag="Pp8")
            nc.gpsimd.tensor_add(p8[:], p4[:, 0:1144], p4[:, 4:1148])
            P10 = per.tile([128, CHUNK], f32, tag="P10")
            nc.gpsimd.tensor_add(P10[:], p8[:, 0:CHUNK], p2[:, 8:CHUNK + 8])

            # d2 = P - S^2/w ; u = 1/sqrt(d2) in bf16 ; us = u*S in bf16
            def make_u(S, P, w, tag):
                ssq_w = per.tile([128, CHUNK], f32, tag=tag + "ssq")
                nc.scalar.activation(ssq_w[:], S[:], ACT.Square,
                                     scale=float(1.0 / np.sqrt(w)))
                d2 = per.tile([128, CHUNK], f32, tag=tag + "d2")
                nc.gpsimd.tensor_tensor(d2[:], P[:], ssq_w[:], ALU.subtract)
                rcp = per.tile([128, CHUNK], f32, tag=tag + "rcp")
                nc.vector.reciprocal_approx_fast(rcp[:], d2[:])
                u = per.tile([128, CHUNK], bf16, tag=tag + "u")
                nc.scalar.activation(u[:], rcp[:], ACT.Sqrt)
                us = per.tile([128, CHUNK], bf16, tag=tag + "us")
                nc.gpsimd.tensor_mul(us[:], u[:], S[:])
                return u, us

            u10, us10 = make_u(S10, P10, W10, "u10")

            p16 = per.tile([128, 1136], f32, tag="Pp16")
            nc.gpsimd.tensor_add(p16[:], p8[:, 0:1136], p8[:, 8:1144])
            P20 = per.tile([128, CHUNK], f32, tag="P20")
            nc.gpsimd.tensor_add(P20[:], p16[:, 0:CHUNK], p4[:, 16:CHUNK + 16])
            u20, us20 = make_u(S20, P20, W20, "u20")

            # ============ tensor: full-T covariance (bf16, replicated) =======
            covp = psum([128, 128], "big")
            for i in range(64):
                blk = xfbs[i // 8][:, (i % 8) * 128:(i % 8 + 1) * 128]
                nc.tensor.matmul(covp[:], lhsT=blk, rhs=blk,
                                 start=(i == 0), stop=(i == 63),
                                 skip_group_check=True)
            cov = per.tile([128, 128], f32, tag="cov")
            nc.vector.tensor_copy(cov[:], covp[:])

            # ============ rolling windows: z matmuls + masked moments ========
            # y[:, 2k] = w20 chunk k, y[:, 2k+1] = w10 chunk k
            r1all = per.tile([128, 16], f32, tag="r1all")
            r2pall = psum([128, 16], "r2p")
            rsqall = per.tile([128, 16], f32, tag="rsqall")

            for k in range(8):
                ksl = slice(k * 128, (k + 1) * 128)
                zp = psum([128, 2 * RW], "zp")
                nc.tensor.matmul(zp[:, 0:RW], lhsT=u20[:, ksl],
                                 rhs=xTb[:, k * 128:k * 128 + RW],
                                 start=True, stop=True, skip_group_check=True)
                nc.tensor.matmul(zp[:, RW:2 * RW], lhsT=u10[:, ksl],
                                 rhs=xTb[:, k * 128:k * 128 + RW],
                                 start=True, stop=True, skip_group_check=True)
                nc.tensor.matmul(r2pall[:, 2 * k:2 * k + 1],
                                 lhsT=us20[:, ksl], rhs=ow20[:],
                                 start=True, stop=True, skip_group_check=True)
                nc.tensor.matmul(r2pall[:, 2 * k + 1:2 * k + 2],
                                 lhsT=us10[:, ksl], rhs=ow10[:],
                                 start=True, stop=True, skip_group_check=True)
                zm = wrk.tile([128, 2 * RW], f32, tag="zm")
                nc.vector.tensor_mul(zm[:], zp[:], mcat)
                V = wrk.tile([128, 2 * RW], f32, tag="V")
                nc.scalar.activation(V[:], zm[:], ACT.Square)
                nc.vector.tensor_reduce(r1all[:, 2 * k:2 * k + 1],
                                        V[:, 0:RW], axis=AX.X, op=ALU.add)
                nc.vector.tensor_reduce(r1all[:, 2 * k + 1:2 * k + 2],
                                        V[:, RW:2 * RW], axis=AX.X,
                                        op=ALU.add)
            nc.scalar.activation(rsqall[:], r2pall[:], ACT.Square)

            # y, finals (batched [128,16])
            y = per.tile([128, 16], f32, tag="y")
            nc.vector.tensor_tensor(y[:], r1all[:], rsqall[:], ALU.subtract)
            c1 = sml.tile([128, 16], f32, tag="c1")
            nc.vector.tensor_tensor(c1[:], y[:], thrv, ALU.is_gt)
            c2 = sml.tile([128, 16], f32, tag="c2")
            nc.vector.tensor_mul(c2[:], c1[:], v20x)
            cnt = sml.tile([128, 1], f32, tag="cnt")
            nc.vector.tensor_reduce(cnt[:], c2[:], axis=AX.X, op=ALU.add)
            nc.vector.tensor_copy(slot(S_COUNT20), psum_scalar(cnt[:])[:])
            hv = sml.tile([128, 16], f32, tag="hv")
            nc.vector.tensor_mul(hv[:], y[:], h10x)
            hs = sml.tile([128, 1], f32, tag="hs")
            nc.vector.tensor_reduce(hs[:], hv[:], axis=AX.X, op=ALU.add)
            nc.vector.tensor_copy(slot(S_HIST10), psum_scalar(hs[:])[:])
            rv = sml.tile([128, 16], f32, tag="rv")
            nc.vector.tensor_mul(rv[:], y[:], r10x)
            rs = sml.tile([128, 1], f32, tag="rs")
            nc.vector.tensor_reduce(rs[:], rv[:], axis=AX.X, op=ALU.add)
            nc.vector.tensor_copy(slot(S_RECENT10), psum_scalar(rs[:])[:])

            # ============ [A,A] post-processing ============
            dscr = wrk.tile([128, 128], f32, tag="dscr")
            nc.vector.tensor_mul(dscr[:], cov[:], ident)
            diag = sml.tile([128, 1], f32, tag="diag")
            nc.vector.tensor_reduce(diag[:], dscr[:], axis=AX.X, op=ALU.add)
            dstd = sml.tile([128, 1], f32, tag="dstd")
            nc.scalar.activation(dstd[:], diag[:], ACT.Sqrt)
            ucol = per.tile([128, 1], f32, tag="ucol")
            nc.vector.reciprocal(ucol[:], dstd[:])
            u2 = sml.tile([128, 1], f32, tag="u2")
            nc.vector.tensor_mul(u2[:], ucol[:], ucol[:])
            du2 = sml.tile([128, 1], f32, tag="du2")
            nc.vector.tensor_mul(du2[:], u2[:], diag[:])
            nc.vector.tensor_copy(slot(S_TRACE), psum_scalar(du2[:])[:])

            uT_p = psum([1, 128], "sc")
            nc.tensor.transpose(uT_p[:], ucol[:], ident)
            uT = per.tile([1, 128], f32, tag="uT")
            nc.vector.tensor_copy(uT[:], uT_p[:])

            def quad_form(mat_sb, out_slot):
                qr = psum([1, 128], "sc")
                nc.tensor.matmul(qr[:], lhsT=ucol[:], rhs=mat_sb,
                                 start=True, stop=True, skip_group_check=True)
                qscr = sml.tile([1, 128], f32, tag="qscr")
                nc.vector.tensor_mul(qscr[:], qr[:], uT[:])
                qacc = sml.tile([1, 1], f32, tag="qacc")
                nc.vector.tensor_reduce(qacc[:], qscr[:], axis=AX.X,
                                        op=ALU.add)
                nc.vector.tensor_copy(out_slot, qacc[:])

            quad_form(cov[:], slot(S_SUMCORR))
            acov = wrk.tile([128, 128], f32, tag="acov")
            nc.scalar.activation(acov[:], cov[:], ACT.Abs)
            quad_form(acov[:], slot(S_SUMABS))

            # corr in bf16 via row-scale, transpose, col-scale
            brow = wrk.tile([128, 128], f32, tag="brow")
            nc.vector.tensor_scalar(brow[:], cov[:], ucol[:], None, ALU.mult)
            bt_p = psum([128, 128], "big")
            nc.tensor.transpose(bt_p[:], brow[:], ident)
            corrb = per.tile([128, 128], bf16, tag="corrb")
            nc.vector.tensor_scalar(corrb[:], bt_p[:], ucol[:], None, ALU.mult)

            # row-layout stats via ones-matmuls, interleaved into the eigen
            # chain's dependency gaps (tensor queue is in-order)
            statp = psum([128, 24], "stat")

            def stat_mms(lo, hi):
                for j in range(lo, hi):
                    nc.tensor.matmul(statp[:, j:j + 1],
                                     lhsT=xTb[:, j * 128:(j + 1) * 128],
                                     rhs=onesb[:], start=True, stop=True,
                                     skip_group_check=True)
                    nc.tensor.matmul(statp[:, 8 + j:9 + j],
                                     lhsT=x2T[:, j * 128:(j + 1) * 128],
                                     rhs=ones[:], start=True, stop=True,
                                     skip_group_check=True)
                    nc.tensor.matmul(statp[:, 16 + j:17 + j],
                                     lhsT=sgT[:, j * 128:(j + 1) * 128],
                                     rhs=onesb[:], start=True, stop=True,
                                     skip_group_check=True)

            # top eigenvalue: 7 bf16 squarings; traces of M^64 and M^128
            M = corrb
            for kk in range(7):
                p = psum([128, 128], "big")
                nc.tensor.matmul(p[:], lhsT=M[:], rhs=M[:],
                                 start=True, stop=True, skip_group_check=True)
                stat_mms(kk, kk + 1)          # fill the cast-wait gap
                if kk >= 5:
                    escr = wrk.tile([128, 128], f32, tag="escr")
                    nc.vector.tensor_mul(escr[:], p[:], ident)
                    edg = sml.tile([128, 1], f32, tag="edg")
                    nc.vector.tensor_reduce(edg[:], escr[:], axis=AX.X,
                                            op=ALU.add)
                    s = S_T6 if kk == 5 else S_T7
                    nc.vector.tensor_copy(slot(s), psum_scalar(edg[:])[:])
                if kk < 6:
                    Mn = wrk.tile([128, 128], bf16, tag="Mn", bufs=2)
                    nc.scalar.activation(Mn[:], p[:], ACT.Copy)
                    M = Mn
            stat_mms(7, 8)

            stats = per.tile([128, 24], f32, tag="stats")
            nc.vector.tensor_copy(stats[:], statp[:])
            sr = stats[:, 0:8]
            ss = stats[:, 8:16]
            rho = stats[:, 16:24]

            # cross-sectional std per t: sqrt((ss - sr^2/A) / (A-1))
            sq2 = sml.tile([128, 8], f32, tag="sq2")
            nc.scalar.activation(sq2[:], sr, ACT.Square,
                                 scale=float(1.0 / np.sqrt(A)))
            varA = sml.tile([128, 8], f32, tag="varA")
            nc.vector.tensor_tensor(varA[:], ss, sq2[:], ALU.subtract)
            csstd = per.tile([128, 8], f32, tag="csstd")
            nc.scalar.activation(csstd[:], varA[:], ACT.Sqrt,
                                 scale=float(1.0 / (A - 1)))
            csr = sml.tile([128, 1], f32, tag="csr")
            nc.vector.tensor_reduce(csr[:], csstd[:], axis=AX.X, op=ALU.add)
            nc.vector.tensor_copy(slot(S_CSSUM), psum_scalar(csr[:])[:])
            nc.vector.tensor_copy(slot(S_CSFIRST), csstd[0:1, 0:1])
            cslast_p = psum([1, 1], "sc")
            nc.tensor.matmul(cslast_p[:], lhsT=cmb[:, OH127_COL:OH127_COL + 1],
                             rhs=csstd[:, 7:8],
                             start=True, stop=True, skip_group_check=True)
            nc.vector.tensor_copy(slot(S_CSLAST), cslast_p[:])

            # ssq partial: sum_t rho_t^2
            rho2 = sml.tile([128, 8], f32, tag="rho2")
            nc.scalar.activation(rho2[:], rho, ACT.Square)
            rr = sml.tile([128, 1], f32, tag="rr")
            nc.vector.tensor_reduce(rr[:], rho2[:], axis=AX.X, op=ALU.add)
            nc.vector.tensor_copy(slot(S_SSQ), psum_scalar(rr[:])[:])

            # ============ position diversity ============
            pa = per.tile([128, 1], f32, tag="pa")
            nc.scalar.activation(pa[:], cmb[:, POS_COL:POS_COL + 1], ACT.Abs)
            nc.vector.tensor_copy(slot(S_PASUM), psum_scalar(pa[:])[:])
            paT_p = psum([1, 128], "sc")
            nc.tensor.transpose(paT_p[:], pa[:], ident)
            paT = sml.tile([1, 128], f32, tag="paT")
            nc.vector.tensor_copy(paT[:], paT_p[:])
            nc.vector.tensor_reduce(slot(S_PAMAX), paT[:], axis=AX.X,
                                    op=ALU.max)

            # ============ herding MLP ============
            h1p = psum([128, 1], "sc")
            nc.tensor.matmul(h1p[:], lhsT=cmb[:, W1A_OFF:W1A_OFF + 128],
                             rhs=cmb[:, XL_COL:XL_COL + 1], start=True,
                             stop=False, skip_group_check=True)
            nc.tensor.matmul(h1p[:], lhsT=cmb[:, W1B_OFF:W1B_OFF + 128],
                             rhs=cmb[:, POS_COL:POS_COL + 1], start=False,
                             stop=True, skip_group_check=True)
            h1 = sml.tile([128, 1], f32, tag="h1")
            nc.scalar.activation(h1[:], h1p[:], ACT.Relu,
                                 bias=cmb[:, B1_COL:B1_COL + 1])
            gk = sml.tile([128, 1], f32, tag="gk")
            nc.vector.tensor_scalar(gk[:], cmb[:, GAM_COL:GAM_COL + 1],
                                    float(1.0 / np.sqrt(1.0 + 1e-5)), None,
                                    ALU.mult)
            h1b = sml.tile([128, 1], f32, tag="h1b")
            nc.vector.tensor_scalar(h1b[:], h1[:], gk[:],
                                    cmb[:, BET_COL:BET_COL + 1],
                                    ALU.mult, ALU.add)
            h2p = psum([64, 1], "sc")
            nc.tensor.matmul(h2p[:], lhsT=cmb[:, W2_OFF:W2_OFF + 64],
                             rhs=h1b[:], start=True, stop=True,
                             skip_group_check=True)
            h2 = sml.tile([64, 1], f32, tag="h2")
            nc.scalar.activation(h2[:], h2p[:], ACT.Relu,
                                 bias=cmb[0:64, B2_COL:B2_COL + 1])
            lg = psum([3, 1], "sc")
            nc.tensor.matmul(lg[:], lhsT=cmb[0:64, W3_OFF:W3_OFF + 3],
                             rhs=h2[:], start=True, stop=True,
                             skip_group_check=True)
            exps = sml.tile([3, 1], f32, tag="exps")
            nc.scalar.activation(exps[:], lg[:], ACT.Exp,
                                 bias=cmb[0:3, B3_COL:B3_COL + 1])
            esum = psum_scalar(exps[:], p=3)
            esum_sb = sml.tile([1, 1], f32, tag="esum_sb")
            nc.vector.tensor_copy(esum_sb[:], esum[:])
            erec = sml.tile([1, 1], f32, tag="erec")
            nc.vector.reciprocal(erec[:], esum_sb[:])
            e2p = psum([1, 1], "sc")
            nc.tensor.matmul(e2p[:], lhsT=cmb[0:3, OH2_COL:OH2_COL + 1],
                             rhs=exps[:], start=True, stop=True,
                             skip_group_check=True)
            e2_sb = sml.tile([1, 1], f32, tag="e2_sb")
            nc.vector.tensor_copy(e2_sb[:], e2p[:])
            nc.vector.tensor_mul(slot(S_SEV), e2_sb[:], erec[:])

            # ============ write out ============
            nc.sync.dma_start(out_d[:, :], out_sb[:])

    nc.compile()
    return nc


def _build_cmb_common(inputs):
    cmb = np.zeros((128, CMB_W), np.float32)
    for j in range(128):
        cmb[j, MCAT_OFF + j:MCAT_OFF + j + W20] = 1.0
        cmb[j, MCAT_OFF + RW + j:MCAT_OFF + RW + j + W10] = 1.0
    cmb[:, THRV_OFF:THRV_OFF + 16:2] = Y_THRESH
    cmb[:, THRV_OFF + 1:THRV_OFF + 16:2] = 1e30
    cmb[:, ID_OFF:ID_OFF + 128] = np.eye(128, dtype=np.float32)
    w1 = np.asarray(inputs["w1"], np.float32)
    cmb[:, W1A_OFF:W1A_OFF + 128] = w1[0:128]
    cmb[:, W1B_OFF:W1B_OFF + 128] = w1[128:256]
    cmb[:, W2_OFF:W2_OFF + 64] = np.asarray(inputs["w2"], np.float32)
    cmb[:, B1_COL] = np.asarray(inputs["b1"], np.float32)
    cmb[:, GAM_COL] = np.asarray(inputs["gamma"], np.float32)
    cmb[:, BET_COL] = np.asarray(inputs["beta"], np.float32)
    cmb[:, POS_COL] = np.asarray(inputs["positions"], np.float32)
    x = np.asarray(inputs["returns_sequence"], np.float32)
    cmb[:, XL_COL] = x[-1]
    cmb[127, OH127_COL] = 1.0
    cmb[0:64, W3_OFF:W3_OFF + 3] = np.asarray(inputs["w3"], np.float32)
    cmb[0:64, B2_COL] = np.asarray(inputs["b2"], np.float32)
    cmb[0:3, B3_COL] = np.asarray(inputs["b3"], np.float32)
    cmb[2, OH2_COL] = 1.0
    return cmb


def _prep_in_maps(inputs):
    import ml_dtypes
    x = np.ascontiguousarray(np.asarray(inputs["returns_sequence"],
                                        dtype=np.float32))
    xfb = np.ascontiguousarray(
        x.reshape(64, 128, 128).transpose(1, 0, 2).reshape(128, T)
        .astype(ml_dtypes.bfloat16))
    cmb_common = _build_cmb_common(inputs)
    in_maps = []
    for c in range(NC_N):
        g = c * CHUNK + np.arange(CHUNK)
        cmb = cmb_common.copy()
        cmb[:, V20X_OFF:V20X_OFF + 16:2] = \
            (g < N20).astype(np.float32).reshape(8, 128).T
        cmb[:, H10X_OFF + 1:H10X_OFF + 16:2] = \
            (g < N10 - 5).astype(np.float32).reshape(8, 128).T
        cmb[:, R10X_OFF + 1:R10X_OFF + 16:2] = \
            ((g >= N10 - 5) & (g < N10)).astype(np.float32).reshape(8, 128).T
        rows = (c * CHUNK + np.arange(XROWS)) % T
        xTc = np.ascontiguousarray(x[rows].T)
        in_maps.append({
            "x_full_b": xfb,
            "xT_chunk": xTc,
            "xTb_chunk": np.ascontiguousarray(xTc.astype(ml_dtypes.bfloat16)),
            "cmb": np.ascontiguousarray(cmb),
        })
    return in_maps


def _combine(per_core):
    count20 = sum(float(per_core[c][0, S_COUNT20]) for c in range(NC_N))
    hist_y = sum(float(per_core[c][0, S_HIST10]) for c in range(NC_N))
    rec_y = sum(float(per_core[c][0, S_RECENT10]) for c in range(NC_N))
    cs_sum = sum(float(per_core[c][0, S_CSSUM]) for c in range(NC_N))
    ssq_sum = sum(float(per_core[c][0, S_SSQ]) for c in range(NC_N))
    cs_first = float(per_core[0][0, S_CSFIRST])
    cs_last = float(per_core[NC_N - 1][0, S_CSLAST])
    r0 = per_core[0][0]
    sum_corr = float(r0[S_SUMCORR])
    sum_abs = float(r0[S_SUMABS])
    trace_c = float(r0[S_TRACE])
    pa_sum = float(r0[S_PASUM])
    pa_max = float(r0[S_PAMAX])
    severity = float(r0[S_SEV])
    t6, t7 = float(r0[S_T6]), float(r0[S_T7])

    phase_locking = count20 / N20
    hist = (hist_y - A * (N10 - 5)) * INV_OD / (N10 - 5)
    recent = (rec_y - A * 5) * INV_OD / 5.0
    surge = 0.0
    if hist > 0:
        surge = min(max((recent - hist) / hist, 0.0), 1.0)
    avg_disp = cs_sum / T
    trend = -(cs_last - cs_first) / (T - 1)
    herding_index = min(max(trend / (avg_disp + 1e-6) + 0.5, 0.0), 1.0)
    avg_corr = (sum_corr - trace_c) * INV_OD
    lam = (t7 / t6) ** (1.0 / 64.0) if t6 > 0 and t7 > 0 else 1.0
    sync_risk = min(1.0, (lam / A) * avg_corr)
    return_div = 1.0 - sum_abs / (A * A)
    pos_div = 1.0 - pa_max / pa_sum
    div_loss = 1.0 - np.sqrt(return_div * pos_div)
    avg_conc = (A * A / 2.0 + ssq_sum / (2.0 * T) - A) / (A * (A - 1))
    phase_coupling = min(max((avg_conc - 0.5) * 2.0, 0.0), 1.0)
    collective = (herding_index + sync_risk + div_loss) / 3.0
    return np.array([herding_index, severity, sync_risk, phase_locking,
                     div_loss, surge, phase_coupling, collective],
                    dtype=np.float32)


def _ensure_ntff_hook():
    """Install the axon NTFF profile hook if the image lacks antenv.axon_hooks."""
    import sys
    import types
    try:
        import antenv.axon_hooks  # noqa: F401
        return True
    except ImportError:
        pass
    try:
        import antenv
        from trn_agent_boot.trn_boot import _ntff_profile_via_ctypes
        mod = types.ModuleType("antenv.axon_hooks")
        state = {}
        mod.set_axon_ntff_profile_hook = lambda h: state.update(h=h)
        mod.get_axon_ntff_profile_hook = lambda: state.get("h")
        sys.modules["antenv.axon_hooks"] = mod
        antenv.axon_hooks = mod
        hook = _ntff_profile_via_ctypes("/opt/axon/libaxon_pjrt.so")
        mod.set_axon_ntff_profile_hook(hook)
        return hook is not None
    except Exception:
        return False


def _run(inputs, trace=False):
    from concourse.bass_utils import run_bass_kernel_spmd
    if trace:
        trace = _ensure_ntff_hook()
    if "nc" not in _PLAN:
        _PLAN["nc"] = _build_program()
    nc = _PLAN["nc"]
    in_maps = _prep_in_maps(inputs)
    res = run_bass_kernel_spmd(nc, in_maps, core_ids=list(range(NC_N)),
                               trace=trace)
    per_core = [res.results[c]["out_vec"] for c in range(NC_N)]
    return _combine(per_core), res


def kernel(**inputs) -> np.ndarray:
    out, _ = _run(inputs, trace=False)
    return out


# revision 15
# speedup vs baseline: 1.8779x; 1.0115x over previous
"""Trainium2 Bass kernel for nn_EmergentRiskMetrics.

Contract: kernel(**inputs) takes the FULL unsharded inputs (as produced by
setup_inputs()) and returns the FULL output (shape [8], float32).

Sharding: data-parallel over the time axis. Each of the 8 cores owns 1024
contiguous rolling-window starts (plus halo) and 1024 rows of the
cross-sectional / sign-concordance statistics; those shard linearly and the
host sums the per-core scalars. The [A,A] covariance work (bf16 matmul
accumulation over all T rows), its nonlinear post-processing (correlation,
|corr| mean, top-eigenvalue power chain) and the tiny MLP are replicated on
every core - a cross-core AllReduce was measured at ~94us fixed cost in
this runtime, far more than recomputing the 64 bf16 matmuls locally.

Engine budget drives the layout: fp32 for all elementwise work on the
Vector/GpSimd engines (bf16 elementwise is ~3x slower there), bf16 only as
matmul operands; row-layout statistics come from ones-matmuls on the
otherwise idle tensor engine; the w=10 rolling pipeline is scheduled before
w=20 so its normalizers are ready the moment the covariance matmuls drain.

Device outputs are per-core scalars; the host only gathers them (sums the
sharded partials, applies final scalar clips/divides) into the 8 outputs.
"""

import numpy as np

T = 8192
A = 128
W20 = 20
W10 = 10
NC_N = 8
CHUNK = 1024            # window starts / owned rows per core
XROWS = 1152            # rolling-scan rows incl. halo (9 x 128)
RW = 160                # z-matmul columns per window chunk (>= 128+w-1)
N20 = T - W20           # 8172 rolling-20 windows
N10 = T - W10           # 8182 rolling-10 windows
OUT_SLOTS = 16
INV_OD = 1.0 / (A * (A - 1))
# rolling > 0.7 in terms of the unnormalized window sum y:
# rolling = (y - A) * INV_OD  =>  y > 0.7 * A*(A-1) + A
Y_THRESH = 0.7 * A * (A - 1) + A

# out_vec slot layout
(S_COUNT20, S_HIST10, S_RECENT10, S_CSSUM, S_CSFIRST, S_CSLAST,
 S_SUMCORR, S_SUMABS, S_TRACE, S_PASUM, S_PAMAX, S_SEV, S_SSQ,
 S_T6, S_T7) = range(15)

# packed-constant column layout (cmb: [128, CMB_W] f32)
MCAT_OFF = 0                      # [128, 2*RW]: w20 mask | w10 mask
THRV_OFF = MCAT_OFF + 2 * RW      # 320  [128,16] interleaved thresholds
V20X_OFF = THRV_OFF + 16          # 336  valid20 on even cols, 0 on odd
H10X_OFF = V20X_OFF + 16          # 352  hist10 on odd cols, 0 on even
R10X_OFF = H10X_OFF + 16          # 368  recent10 on odd cols, 0 on even
ID_OFF = R10X_OFF + 16            # 384
W1A_OFF = ID_OFF + 128            # 512
W1B_OFF = W1A_OFF + 128           # 640
W2_OFF = W1B_OFF + 128            # 768
B1_COL = W2_OFF + 64              # 832
GAM_COL = B1_COL + 1              # 833
BET_COL = GAM_COL + 1             # 834
POS_COL = BET_COL + 1             # 835
XL_COL = POS_COL + 1              # 836
OH127_COL = XL_COL + 1            # 837
W3_OFF = OH127_COL + 1            # 838 (3 cols, rows 0:64)
B2_COL = W3_OFF + 3               # 841 (rows 0:64)
B3_COL = B2_COL + 1               # 842 (rows 0:3)
OH2_COL = B3_COL + 1              # 843 (rows 0:3)
CMB_W = OH2_COL + 1               # 844

_PLAN = {}


def _build_program():
    import concourse.bacc as bacc
    import concourse.tile as tile
    from concourse import mybir

    f32 = mybir.dt.float32
    bf16 = mybir.dt.bfloat16
    ALU = mybir.AluOpType
    ACT = mybir.ActivationFunctionType
    AX = mybir.AxisListType

    nc = bacc.Bacc("TRN2", target_bir_lowering=False, debug=False,
                   num_devices=NC_N)

    xfb_in = nc.dram_tensor("x_full_b", [128, T], bf16,
                            kind="ExternalInput").ap()
    xTb_in = nc.dram_tensor("xTb_chunk", [128, XROWS], bf16,
                            kind="ExternalInput").ap()
    cmb_in = nc.dram_tensor("cmb", [128, CMB_W], f32,
                            kind="ExternalInput").ap()
    out_d = nc.dram_tensor("out_vec", [1, OUT_SLOTS], f32,
                           kind="ExternalOutput").ap()

    with tile.TileContext(nc) as tc:
        with tc.tile_pool(name="const", bufs=1) as cst, \
             tc.tile_pool(name="persist", bufs=1) as per, \
             tc.tile_pool(name="work", bufs=3) as wrk, \
             tc.tile_pool(name="small", bufs=6) as sml, \
             tc.tile_pool(name="ps", bufs=1, space="PSUM") as ps:

            psum_bufs = {"zp": 2, "big": 2, "sc": 1, "r2p": 1, "stat": 1}

            def psum(shape, tag):
                return ps.tile(shape, f32, tag=tag, name=tag,
                               bufs=psum_bufs[tag])

            # ---- input DMAs: few, large, on the two hardware-DGE queues
            # (sync/SP and scalar/Activation); gpsimd's software DGE is slow
            xTb = cst.tile([128, XROWS], bf16, tag="xTb")
            nc.sync.dma_start(xTb[:], xTb_in[:, :])
            xfa = cst.tile([128, 4 * CHUNK], bf16, tag="xfa")
            nc.sync.dma_start(xfa[:], xfb_in[:, 0:4 * CHUNK])
            cmb = cst.tile([128, CMB_W], f32, tag="cmb")
            nc.scalar.dma_start(cmb[:], cmb_in[:, :])
            xfb2 = cst.tile([128, 4 * CHUNK], bf16, tag="xfb2")
            nc.scalar.dma_start(xfb2[:], xfb_in[:, 4 * CHUNK:T])
            # f32 copy of the column-layout x for the DVE chains
            xT = per.tile([128, XROWS], f32, tag="xT")
            nc.scalar.activation(xT[:], xTb[:], ACT.Copy)

            mcat = cmb[:, MCAT_OFF:MCAT_OFF + 2 * RW]
            thrv = cmb[:, THRV_OFF:THRV_OFF + 16]
            v20x = cmb[:, V20X_OFF:V20X_OFF + 16]
            h10x = cmb[:, H10X_OFF:H10X_OFF + 16]
            r10x = cmb[:, R10X_OFF:R10X_OFF + 16]
            ident = cmb[:, ID_OFF:ID_OFF + 128]

            ones = cst.tile([128, 1], f32, tag="ones")
            nc.vector.memset(ones[:], 1.0)
            onesb = cst.tile([128, 1], bf16, tag="onesb")
            nc.vector.memset(onesb[:], 1.0)
            ow20 = cst.tile([128, 1], bf16, tag="ow20")
            nc.vector.memset(ow20[:], float(1.0 / np.sqrt(W20)))
            ow10 = cst.tile([128, 1], bf16, tag="ow10")
            nc.vector.memset(ow10[:], float(1.0 / np.sqrt(W10)))

            out_sb = per.tile([1, OUT_SLOTS], f32, tag="out_sb")
            nc.vector.memset(out_sb[:], 0.0)

            def slot(i):
                return out_sb[:, i:i + 1]

            def psum_scalar(vec_sb, p=128):
                o = psum([1, 1], "sc")
                lhs = ones[0:p, :] if p != 128 else ones[:]
                nc.tensor.matmul(o[:], lhsT=lhs, rhs=vec_sb,
                                 start=True, stop=True, skip_group_check=True)
                return o

            # ============ scalar: squares / signs feeding the chains =========
            x2T = per.tile([128, 1151], f32, tag="x2T")
            nc.scalar.activation(x2T[:], xT[:, 0:1151], ACT.Square)
            sgT = per.tile([128, CHUNK], bf16, tag="sgT")
            nc.scalar.activation(sgT[:], xTb[:, 0:CHUNK], ACT.Sign)

            # ============ rolling sums (S on vector, P=sum x^2 on gpsimd) ====
            # w=10 parts first so its normalizer is ready when cov drains
            s2 = per.tile([128, 1151], f32, tag="Ss2")
            nc.vector.tensor_add(s2[:], xT[:, 0:1151], xT[:, 1:1152])
            s4 = per.tile([128, 1149], f32, tag="Ss4")
            nc.vector.tensor_add(s4[:], s2[:, 0:1149], s2[:, 2:1151])
            s8 = per.tile([128, 1145], f32, tag="Ss8")
            nc.vector.tensor_add(s8[:], s4[:, 0:1145], s4[:, 4:1149])
            S10 = per.tile([128, CHUNK], f32, tag="S10")
            nc.vector.tensor_add(S10[:], s8[:, 0:CHUNK], s2[:, 8:CHUNK + 8])
            s16 = per.tile([128, 1137], f32, tag="Ss16")
            nc.vector.tensor_add(s16[:], s8[:, 0:1137], s8[:, 8:1145])
            S20 = per.tile([128, CHUNK], f32, tag="S20")
            nc.vector.tensor_add(S20[:], s16[:, 0:CHUNK], s4[:, 16:CHUNK + 16])

            p2 = per.tile([128, 1150], f32, tag="Pp2")
            nc.gpsimd.tensor_add(p2[:], x2T[:, 0:1150], x2T[:, 1:1151])
            p4 = per.tile([128, 1148], f32, tag="Pp4")
            nc.gpsimd.tensor_add(p4[:], p2[:, 0:1148], p2[:, 2:1150])
            p8 = per.tile([128, 1144], f32, tag="Pp8")
            nc.gpsimd.tensor_add(p8[:], p4[:, 0:1144], p4[:, 4:1148])
            P10 = per.tile([128, CHUNK], f32, tag="P10")
            nc.gpsimd.tensor_add(P10[:], p8[:, 0:CHUNK], p2[:, 8:CHUNK + 8])

            # d2 = P - S^2/w ; u = 1/sqrt(d2) in bf16 ; us = u*S in bf16
            def make_u(S, P, w, tag):
                ssq_w = per.tile([128, CHUNK], f32, tag=tag + "ssq")
                nc.scalar.activation(ssq_w[:], S[:], ACT.Square,
                                     scale=float(1.0 / np.sqrt(w)))
                d2 = per.tile([128, CHUNK], f32, tag=tag + "d2")
                nc.gpsimd.tensor_tensor(d2[:], P[:], ssq_w[:], ALU.subtract)
                rcp = per.tile([128, CHUNK], f32, tag=tag + "rcp")
                nc.vector.reciprocal_approx_fast(rcp[:], d2[:])
                u = per.tile([128, CHUNK], bf16, tag=tag + "u")
                nc.scalar.activation(u[:], rcp[:], ACT.Sqrt)
                us = per.tile([128, CHUNK], bf16, tag=tag + "us")
                nc.gpsimd.tensor_mul(us[:], u[:], S[:])
                return u, us

            u10, us10 = make_u(S10, P10, W10, "u10")

            p16 = per.tile([128, 1136], f32, tag="Pp16")
            nc.gpsimd.tensor_add(p16[:], p8[:, 0:1136], p8[:, 8:1144])
            P20 = per.tile([128, CHUNK], f32, tag="P20")
            nc.gpsimd.tensor_add(P20[:], p16[:, 0:CHUNK], p4[:, 16:CHUNK + 16])
            u20, us20 = make_u(S20, P20, W20, "u20")

            # ============ tensor: full-T covariance (bf16, replicated) =======
            covp = psum([128, 128], "big")
            for i in range(64):
                src = xfa if i < 32 else xfb2
                o = (i % 32) * 128
                blk = src[:, o:o + 128]
                nc.tensor.matmul(covp[:], lhsT=blk, rhs=blk,
                                 start=(i == 0), stop=(i == 63),
                                 skip_group_check=True)
            cov = per.tile([128, 128], f32, tag="cov")
            nc.vector.tensor_copy(cov[:], covp[:])

            # ============ rolling windows: z matmuls + masked moments ========
            # y[:, 2k] = w20 chunk k, y[:, 2k+1] = w10 chunk k
            r1all = per.tile([128, 16], f32, tag="r1all")
            r2pall = psum([128, 16], "r2p")
            rsqall = per.tile([128, 16], f32, tag="rsqall")

            for k in range(8):
                ksl = slice(k * 128, (k + 1) * 128)
                zp = psum([128, 2 * RW], "zp")
                nc.tensor.matmul(zp[:, 0:RW], lhsT=u20[:, ksl],
                                 rhs=xTb[:, k * 128:k * 128 + RW],
                                 start=True, stop=True, skip_group_check=True)
                nc.tensor.matmul(zp[:, RW:2 * RW], lhsT=u10[:, ksl],
                                 rhs=xTb[:, k * 128:k * 128 + RW],
                                 start=True, stop=True, skip_group_check=True)
                nc.tensor.matmul(r2pall[:, 2 * k:2 * k + 1],
                                 lhsT=us20[:, ksl], rhs=ow20[:],
                                 start=True, stop=True, skip_group_check=True)
                nc.tensor.matmul(r2pall[:, 2 * k + 1:2 * k + 2],
                                 lhsT=us10[:, ksl], rhs=ow10[:],
                                 start=True, stop=True, skip_group_check=True)
                zm = wrk.tile([128, 2 * RW], f32, tag="zm")
                nc.vector.tensor_mul(zm[:], zp[:], mcat)
                V = wrk.tile([128, 2 * RW], f32, tag="V")
                nc.scalar.activation(V[:], zm[:], ACT.Square)
                nc.vector.tensor_reduce(r1all[:, 2 * k:2 * k + 1],
                                        V[:, 0:RW], axis=AX.X, op=ALU.add)
                nc.vector.tensor_reduce(r1all[:, 2 * k + 1:2 * k + 2],
                                        V[:, RW:2 * RW], axis=AX.X,
                                        op=ALU.add)
            nc.scalar.activation(rsqall[:], r2pall[:], ACT.Square)

            # y, finals (batched [128,16])
            y = per.tile([128, 16], f32, tag="y")
            nc.vector.tensor_tensor(y[:], r1all[:], rsqall[:], ALU.subtract)
            c1 = sml.tile([128, 16], f32, tag="c1")
            nc.vector.tensor_tensor(c1[:], y[:], thrv, ALU.is_gt)
            c2 = sml.tile([128, 16], f32, tag="c2")
            nc.vector.tensor_mul(c2[:], c1[:], v20x)
            cnt = sml.tile([128, 1], f32, tag="cnt")
            nc.vector.tensor_reduce(cnt[:], c2[:], axis=AX.X, op=ALU.add)
            nc.vector.tensor_copy(slot(S_COUNT20), psum_scalar(cnt[:])[:])
            hv = sml.tile([128, 16], f32, tag="hv")
            nc.vector.tensor_mul(hv[:], y[:], h10x)
            hs = sml.tile([128, 1], f32, tag="hs")
            nc.vector.tensor_reduce(hs[:], hv[:], axis=AX.X, op=ALU.add)
            nc.vector.tensor_copy(slot(S_HIST10), psum_scalar(hs[:])[:])
            rv = sml.tile([128, 16], f32, tag="rv")
            nc.vector.tensor_mul(rv[:], y[:], r10x)
            rs = sml.tile([128, 1], f32, tag="rs")
            nc.vector.tensor_reduce(rs[:], rv[:], axis=AX.X, op=ALU.add)
            nc.vector.tensor_copy(slot(S_RECENT10), psum_scalar(rs[:])[:])

            # ============ [A,A] post-processing ============
            dscr = wrk.tile([128, 128], f32, tag="dscr")
            nc.vector.tensor_mul(dscr[:], cov[:], ident)
            diag = sml.tile([128, 1], f32, tag="diag")
            nc.vector.tensor_reduce(diag[:], dscr[:], axis=AX.X, op=ALU.add)
            dstd = sml.tile([128, 1], f32, tag="dstd")
            nc.scalar.activation(dstd[:], diag[:], ACT.Sqrt)
            ucol = per.tile([128, 1], f32, tag="ucol")
            nc.vector.reciprocal(ucol[:], dstd[:])
            u2 = sml.tile([128, 1], f32, tag="u2")
            nc.vector.tensor_mul(u2[:], ucol[:], ucol[:])
            du2 = sml.tile([128, 1], f32, tag="du2")
            nc.vector.tensor_mul(du2[:], u2[:], diag[:])
            nc.vector.tensor_copy(slot(S_TRACE), psum_scalar(du2[:])[:])

            uT_p = psum([1, 128], "sc")
            nc.tensor.transpose(uT_p[:], ucol[:], ident)
            uT = per.tile([1, 128], f32, tag="uT")
            nc.vector.tensor_copy(uT[:], uT_p[:])

            def quad_form(mat_sb, out_slot):
                qr = psum([1, 128], "sc")
                nc.tensor.matmul(qr[:], lhsT=ucol[:], rhs=mat_sb,
                                 start=True, stop=True, skip_group_check=True)
                qscr = sml.tile([1, 128], f32, tag="qscr")
                nc.vector.tensor_mul(qscr[:], qr[:], uT[:])
                qacc = sml.tile([1, 1], f32, tag="qacc")
                nc.vector.tensor_reduce(qacc[:], qscr[:], axis=AX.X,
                                        op=ALU.add)
                nc.vector.tensor_copy(out_slot, qacc[:])

            quad_form(cov[:], slot(S_SUMCORR))
            acov = wrk.tile([128, 128], f32, tag="acov")
            nc.scalar.activation(acov[:], cov[:], ACT.Abs)
            quad_form(acov[:], slot(S_SUMABS))

            # corr in bf16 via row-scale, transpose, col-scale
            brow = wrk.tile([128, 128], f32, tag="brow")
            nc.vector.tensor_scalar(brow[:], cov[:], ucol[:], None, ALU.mult)
            bt_p = psum([128, 128], "big")
            nc.tensor.transpose(bt_p[:], brow[:], ident)
            corrb = per.tile([128, 128], bf16, tag="corrb")
            nc.vector.tensor_scalar(corrb[:], bt_p[:], ucol[:], None, ALU.mult)

            # row-layout stats via ones-matmuls, interleaved into the eigen
            # chain's dependency gaps (tensor queue is in-order)
            statp = psum([128, 24], "stat")

            def stat_mms(lo, hi):
                for j in range(lo, hi):
                    nc.tensor.matmul(statp[:, j:j + 1],
                                     lhsT=xTb[:, j * 128:(j + 1) * 128],
                                     rhs=onesb[:], start=True, stop=True,
                                     skip_group_check=True)
                    nc.tensor.matmul(statp[:, 8 + j:9 + j],
                                     lhsT=x2T[:, j * 128:(j + 1) * 128],
                                     rhs=ones[:], start=True, stop=True,
                                     skip_group_check=True)
                    nc.tensor.matmul(statp[:, 16 + j:17 + j],
                                     lhsT=sgT[:, j * 128:(j + 1) * 128],
                                     rhs=onesb[:], start=True, stop=True,
                                     skip_group_check=True)

            # top eigenvalue: 7 bf16 squarings; traces of M^64 and M^128
            M = corrb
            for kk in range(7):
                p = psum([128, 128], "big")
                nc.tensor.matmul(p[:], lhsT=M[:], rhs=M[:],
                                 start=True, stop=True, skip_group_check=True)
                stat_mms(kk, kk + 1)          # fill the cast-wait gap
                if kk >= 5:
                    escr = wrk.tile([128, 128], f32, tag="escr")
                    nc.vector.tensor_mul(escr[:], p[:], ident)
                    edg = sml.tile([128, 1], f32, tag="edg")
                    nc.vector.tensor_reduce(edg[:], escr[:], axis=AX.X,
                                            op=ALU.add)
                    s = S_T6 if kk == 5 else S_T7
                    nc.vector.tensor_copy(slot(s), psum_scalar(edg[:])[:])
                if kk < 6:
                    Mn = wrk.tile([128, 128], bf16, tag="Mn", bufs=2)
                    nc.scalar.activation(Mn[:], p[:], ACT.Copy)
                    M = Mn
            stat_mms(7, 8)

            stats = per.tile([128, 24], f32, tag="stats")
            nc.vector.tensor_copy(stats[:], statp[:])
            sr = stats[:, 0:8]
            ss = stats[:, 8:16]
            rho = stats[:, 16:24]

            # cross-sectional std per t: sqrt((ss - sr^2/A) / (A-1))
            sq2 = sml.tile([128, 8], f32, tag="sq2")
            nc.scalar.activation(sq2[:], sr, ACT.Square,
                                 scale=float(1.0 / np.sqrt(A)))
            varA = sml.tile([128, 8], f32, tag="varA")
            nc.vector.tensor_tensor(varA[:], ss, sq2[:], ALU.subtract)
            csstd = per.tile([128, 8], f32, tag="csstd")
            nc.scalar.activation(csstd[:], varA[:], ACT.Sqrt,
                                 scale=float(1.0 / (A - 1)))
            csr = sml.tile([128, 1], f32, tag="csr")
            nc.vector.tensor_reduce(csr[:], csstd[:], axis=AX.X, op=ALU.add)
            nc.vector.tensor_copy(slot(S_CSSUM), psum_scalar(csr[:])[:])
            nc.vector.tensor_copy(slot(S_CSFIRST), csstd[0:1, 0:1])
            cslast_p = psum([1, 1], "sc")
            nc.tensor.matmul(cslast_p[:], lhsT=cmb[:, OH127_COL:OH127_COL + 1],
                             rhs=csstd[:, 7:8],
                             start=True, stop=True, skip_group_check=True)
            nc.vector.tensor_copy(slot(S_CSLAST), cslast_p[:])

            # ssq partial: sum_t rho_t^2
            rho2 = sml.tile([128, 8], f32, tag="rho2")
            nc.scalar.activation(rho2[:], rho, ACT.Square)
            rr = sml.tile([128, 1], f32, tag="rr")
            nc.vector.tensor_reduce(rr[:], rho2[:], axis=AX.X, op=ALU.add)
            nc.vector.tensor_copy(slot(S_SSQ), psum_scalar(rr[:])[:])

            # ============ position diversity ============
            pa = per.tile([128, 1], f32, tag="pa")
            nc.scalar.activation(pa[:], cmb[:, POS_COL:POS_COL + 1], ACT.Abs)
            nc.vector.tensor_copy(slot(S_PASUM), psum_scalar(pa[:])[:])
            paT_p = psum([1, 128], "sc")
            nc.tensor.transpose(paT_p[:], pa[:], ident)
            paT = sml.tile([1, 128], f32, tag="paT")
            nc.vector.tensor_copy(paT[:], paT_p[:])
            nc.vector.tensor_reduce(slot(S_PAMAX), paT[:], axis=AX.X,
                                    op=ALU.max)

            # ============ herding MLP ============
            h1p = psum([128, 1], "sc")
            nc.tensor.matmul(h1p[:], lhsT=cmb[:, W1A_OFF:W1A_OFF + 128],
                             rhs=cmb[:, XL_COL:XL_COL + 1], start=True,
                             stop=False, skip_group_check=True)
            nc.tensor.matmul(h1p[:], lhsT=cmb[:, W1B_OFF:W1B_OFF + 128],
                             rhs=cmb[:, POS_COL:POS_COL + 1], start=False,
                             stop=True, skip_group_check=True)
            h1 = sml.tile([128, 1], f32, tag="h1")
            nc.scalar.activation(h1[:], h1p[:], ACT.Relu,
                                 bias=cmb[:, B1_COL:B1_COL + 1])
            gk = sml.tile([128, 1], f32, tag="gk")
            nc.vector.tensor_scalar(gk[:], cmb[:, GAM_COL:GAM_COL + 1],
                                    float(1.0 / np.sqrt(1.0 + 1e-5)), None,
                                    ALU.mult)
            h1b = sml.tile([128, 1], f32, tag="h1b")
            nc.vector.tensor_scalar(h1b[:], h1[:], gk[:],
                                    cmb[:, BET_COL:BET_COL + 1],
                                    ALU.mult, ALU.add)
            h2p = psum([64, 1], "sc")
            nc.tensor.matmul(h2p[:], lhsT=cmb[:, W2_OFF:W2_OFF + 64],
                             rhs=h1b[:], start=True, stop=True,
                             skip_group_check=True)
            h2 = sml.tile([64, 1], f32, tag="h2")
            nc.scalar.activation(h2[:], h2p[:], ACT.Relu,
                                 bias=cmb[0:64, B2_COL:B2_COL + 1])
            lg = psum([3, 1], "sc")
            nc.tensor.matmul(lg[:], lhsT=cmb[0:64, W3_OFF:W3_OFF + 3],
                             rhs=h2[:], start=True, stop=True,
                             skip_group_check=True)
            exps = sml.tile([3, 1], f32, tag="exps")
            nc.scalar.activation(exps[:], lg[:], ACT.Exp,
                                 bias=cmb[0:3, B3_COL:B3_COL + 1])
            esum = psum_scalar(exps[:], p=3)
            esum_sb = sml.tile([1, 1], f32, tag="esum_sb")
            nc.vector.tensor_copy(esum_sb[:], esum[:])
            erec = sml.tile([1, 1], f32, tag="erec")
            nc.vector.reciprocal(erec[:], esum_sb[:])
            e2p = psum([1, 1], "sc")
            nc.tensor.matmul(e2p[:], lhsT=cmb[0:3, OH2_COL:OH2_COL + 1],
                             rhs=exps[:], start=True, stop=True,
                             skip_group_check=True)
            e2_sb = sml.tile([1, 1], f32, tag="e2_sb")
            nc.vector.tensor_copy(e2_sb[:], e2p[:])
            nc.vector.tensor_mul(slot(S_SEV), e2_sb[:], erec[:])

            # ============ write out ============
            nc.sync.dma_start(out_d[:, :], out_sb[:])

    nc.compile()
    return nc


def _build_cmb_common(inputs):
    cmb = np.zeros((128, CMB_W), np.float32)
    for j in range(128):
        cmb[j, MCAT_OFF + j:MCAT_OFF + j + W20] = 1.0
        cmb[j, MCAT_OFF + RW + j:MCAT_OFF + RW + j + W10] = 1.0
    cmb[:, THRV_OFF:THRV_OFF + 16:2] = Y_THRESH
    cmb[:, THRV_OFF + 1:THRV_OFF + 16:2] = 1e30
    cmb[:, ID_OFF:ID_OFF + 128] = np.eye(128, dtype=np.float32)
    w1 = np.asarray(inputs["w1"], np.float32)
    cmb[:, W1A_OFF:W1A_OFF + 128] = w1[0:128]
    cmb[:, W1B_OFF:W1B_OFF + 128] = w1[128:256]
    cmb[:, W2_OFF:W2_OFF + 64] = np.asarray(inputs["w2"], np.float32)
    cmb[:, B1_COL] = np.asarray(inputs["b1"], np.float32)
    cmb[:, GAM_COL] = np.asarray(inputs["gamma"], np.float32)
    cmb[:, BET_COL] = np.asarray(inputs["beta"], np.float32)
    cmb[:, POS_COL] = np.asarray(inputs["positions"], np.float32)
    x = np.asarray(inputs["returns_sequence"], np.float32)
    cmb[:, XL_COL] = x[-1]
    cmb[127, OH127_COL] = 1.0
    cmb[0:64, W3_OFF:W3_OFF + 3] = np.asarray(inputs["w3"], np.float32)
    cmb[0:64, B2_COL] = np.asarray(inputs["b2"], np.float32)
    cmb[0:3, B3_COL] = np.asarray(inputs["b3"], np.float32)
    cmb[2, OH2_COL] = 1.0
    return cmb


def _prep_in_maps(inputs):
    import ml_dtypes
    x = np.ascontiguousarray(np.asarray(inputs["returns_sequence"],
                                        dtype=np.float32))
    xfb = np.ascontiguousarray(
        x.reshape(64, 128, 128).transpose(1, 0, 2).reshape(128, T)
        .astype(ml_dtypes.bfloat16))
    cmb_common = _build_cmb_common(inputs)
    in_maps = []
    for c in range(NC_N):
        g = c * CHUNK + np.arange(CHUNK)
        cmb = cmb_common.copy()
        cmb[:, V20X_OFF:V20X_OFF + 16:2] = \
            (g < N20).astype(np.float32).reshape(8, 128).T
        cmb[:, H10X_OFF + 1:H10X_OFF + 16:2] = \
            (g < N10 - 5).astype(np.float32).reshape(8, 128).T
        cmb[:, R10X_OFF + 1:R10X_OFF + 16:2] = \
            ((g >= N10 - 5) & (g < N10)).astype(np.float32).reshape(8, 128).T
        rows = (c * CHUNK + np.arange(XROWS)) % T
        xTc = np.ascontiguousarray(x[rows].T)
        in_maps.append({
            "x_full_b": xfb,
            "xTb_chunk": np.ascontiguousarray(xTc.astype(ml_dtypes.bfloat16)),
            "cmb": np.ascontiguousarray(cmb),
        })
    return in_maps


def _combine(per_core):
    count20 = sum(float(per_core[c][0, S_COUNT20]) for c in range(NC_N))
    hist_y = sum(float(per_core[c][0, S_HIST10]) for c in range(NC_N))
    rec_y = sum(float(per_core[c][0, S_RECENT10]) for c in range(NC_N))
    cs_sum = sum(float(per_core[c][0, S_CSSUM]) for c in range(NC_N))
    ssq_sum = sum(float(per_core[c][0, S_SSQ]) for c in range(NC_N))
    cs_first = float(per_core[0][0, S_CSFIRST])
    cs_last = float(per_core[NC_N - 1][0, S_CSLAST])
    r0 = per_core[0][0]
    sum_corr = float(r0[S_SUMCORR])
    sum_abs = float(r0[S_SUMABS])
    trace_c = float(r0[S_TRACE])
    pa_sum = float(r0[S_PASUM])
    pa_max = float(r0[S_PAMAX])
    severity = float(r0[S_SEV])
    t6, t7 = float(r0[S_T6]), float(r0[S_T7])

    phase_locking = count20 / N20
    hist = (hist_y - A * (N10 - 5)) * INV_OD / (N10 - 5)
    recent = (rec_y - A * 5) * INV_OD / 5.0
    surge = 0.0
    if hist > 0:
        surge = min(max((recent - hist) / hist, 0.0), 1.0)
    avg_disp = cs_sum / T
    trend = -(cs_last - cs_first) / (T - 1)
    herding_index = min(max(trend / (avg_disp + 1e-6) + 0.5, 0.0), 1.0)
    avg_corr = (sum_corr - trace_c) * INV_OD
    lam = (t7 / t6) ** (1.0 / 64.0) if t6 > 0 and t7 > 0 else 1.0
    sync_risk = min(1.0, (lam / A) * avg_corr)
    return_div = 1.0 - sum_abs / (A * A)
    pos_div = 1.0 - pa_max / pa_sum
    div_loss = 1.0 - np.sqrt(return_div * pos_div)
    avg_conc = (A * A / 2.0 + ssq_sum / (2.0 * T) - A) / (A * (A - 1))
    phase_coupling = min(max((avg_conc - 0.5) * 2.0, 0.0), 1.0)
    collective = (herding_index + sync_risk + div_loss) / 3.0
    return np.array([herding_index, severity, sync_risk, phase_locking,
                     div_loss, surge, phase_coupling, collective],
                    dtype=np.float32)


def _ensure_ntff_hook():
    """Install the axon NTFF profile hook if the image lacks antenv.axon_hooks."""
    import sys
    import types
    try:
        import antenv.axon_hooks  # noqa: F401
        return True
    except ImportError:
        pass
    try:
        import antenv
        from trn_agent_boot.trn_boot import _ntff_profile_via_ctypes
        mod = types.ModuleType("antenv.axon_hooks")
        state = {}
        mod.set_axon_ntff_profile_hook = lambda h: state.update(h=h)
        mod.get_axon_ntff_profile_hook = lambda: state.get("h")
        sys.modules["antenv.axon_hooks"] = mod
        antenv.axon_hooks = mod
        hook = _ntff_profile_via_ctypes("/opt/axon/libaxon_pjrt.so")
        mod.set_axon_ntff_profile_hook(hook)
        return hook is not None
    except Exception:
        return False


def _run(inputs, trace=False):
    from concourse.bass_utils import run_bass_kernel_spmd
    if trace:
        trace = _ensure_ntff_hook()
    if "nc" not in _PLAN:
        _PLAN["nc"] = _build_program()
    nc = _PLAN["nc"]
    in_maps = _prep_in_maps(inputs)
    res = run_bass_kernel_spmd(nc, in_maps, core_ids=list(range(NC_N)),
                               trace=trace)
    per_core = [res.results[c]["out_vec"] for c in range(NC_N)]
    return _combine(per_core), res


def kernel(**inputs) -> np.ndarray:
    out, _ = _run(inputs, trace=False)
    return out


# revision 16
# speedup vs baseline: 1.8829x; 1.0027x over previous
"""Trainium2 Bass kernel for nn_EmergentRiskMetrics.

Contract: kernel(**inputs) takes the FULL unsharded inputs (as produced by
setup_inputs()) and returns the FULL output (shape [8], float32).

Sharding: data-parallel over the time axis. Each of the 8 cores owns 1024
contiguous rolling-window starts (plus halo) and 1024 rows of the
cross-sectional / sign-concordance statistics; those shard linearly and the
host sums the per-core scalars. The [A,A] covariance work (bf16 matmul
accumulation over all T rows), its nonlinear post-processing (correlation,
|corr| mean, top-eigenvalue power chain) and the tiny MLP are replicated on
every core - a cross-core AllReduce was measured at ~94us fixed cost in
this runtime, far more than recomputing the 64 bf16 matmuls locally.

Engine budget drives the layout: fp32 for all elementwise work on the
Vector/GpSimd engines (bf16 elementwise is ~3x slower there), bf16 only as
matmul operands; row-layout statistics come from ones-matmuls on the
otherwise idle tensor engine; the w=10 rolling pipeline is scheduled before
w=20 so its normalizers are ready the moment the covariance matmuls drain.

Device outputs are per-core scalars; the host only gathers them (sums the
sharded partials, applies final scalar clips/divides) into the 8 outputs.
"""

import numpy as np

T = 8192
A = 128
W20 = 20
W10 = 10
NC_N = 8
CHUNK = 1024            # window starts / owned rows per core
XROWS = 1152            # rolling-scan rows incl. halo (9 x 128)
RW = 160                # z-matmul columns per window chunk (>= 128+w-1)
N20 = T - W20           # 8172 rolling-20 windows
N10 = T - W10           # 8182 rolling-10 windows
OUT_SLOTS = 16
INV_OD = 1.0 / (A * (A - 1))
# rolling > 0.7 in terms of the unnormalized window sum y:
# rolling = (y - A) * INV_OD  =>  y > 0.7 * A*(A-1) + A
Y_THRESH = 0.7 * A * (A - 1) + A

# out_vec slot layout
(S_COUNT20, S_HIST10, S_RECENT10, S_CSSUM, S_CSFIRST, S_CSLAST,
 S_SUMCORR, S_SUMABS, S_TRACE, S_PASUM, S_PAMAX, S_SEV, S_SSQ,
 S_T6, S_T7) = range(15)

# packed-constant column layout (cmb: [128, CMB_W] f32)
MCAT_OFF = 0                      # [128, 2*RW]: w20 mask | w10 mask
THRV_OFF = MCAT_OFF + 2 * RW      # 320  [128,16] interleaved thresholds
V20X_OFF = THRV_OFF + 16          # 336  valid20 on even cols, 0 on odd
H10X_OFF = V20X_OFF + 16          # 352  hist10 on odd cols, 0 on even
R10X_OFF = H10X_OFF + 16          # 368  recent10 on odd cols, 0 on even
ID_OFF = R10X_OFF + 16            # 384
W1A_OFF = ID_OFF + 128            # 512
W1B_OFF = W1A_OFF + 128           # 640
W2_OFF = W1B_OFF + 128            # 768
B1_COL = W2_OFF + 64              # 832
GAM_COL = B1_COL + 1              # 833
BET_COL = GAM_COL + 1             # 834
POS_COL = BET_COL + 1             # 835
XL_COL = POS_COL + 1              # 836
OH127_COL = XL_COL + 1            # 837
W3_OFF = OH127_COL + 1            # 838 (3 cols, rows 0:64)
B2_COL = W3_OFF + 3               # 841 (rows 0:64)
B3_COL = B2_COL + 1               # 842 (rows 0:3)
OH2_COL = B3_COL + 1              # 843 (rows 0:3)
CMB_W = OH2_COL + 1               # 844

_PLAN = {}


def _build_program():
    import concourse.bacc as bacc
    import concourse.tile as tile
    from concourse import mybir

    f32 = mybir.dt.float32
    bf16 = mybir.dt.bfloat16
    ALU = mybir.AluOpType
    ACT = mybir.ActivationFunctionType
    AX = mybir.AxisListType

    nc = bacc.Bacc("TRN2", target_bir_lowering=False, debug=False,
                   num_devices=NC_N)

    xfb_in = nc.dram_tensor("x_full_b", [128, T], bf16,
                            kind="ExternalInput").ap()
    xTb_in = nc.dram_tensor("xTb_chunk", [128, XROWS], bf16,
                            kind="ExternalInput").ap()
    cmb_in = nc.dram_tensor("cmb", [128, CMB_W], f32,
                            kind="ExternalInput").ap()
    out_d = nc.dram_tensor("out_vec", [1, OUT_SLOTS], f32,
                           kind="ExternalOutput").ap()

    with tile.TileContext(nc) as tc:
        with tc.tile_pool(name="const", bufs=1) as cst, \
             tc.tile_pool(name="persist", bufs=1) as per, \
             tc.tile_pool(name="work", bufs=3) as wrk, \
             tc.tile_pool(name="small", bufs=6) as sml, \
             tc.tile_pool(name="ps", bufs=1, space="PSUM") as ps:

            psum_bufs = {"zp": 2, "big": 2, "sc": 1, "r2p": 1, "stat": 1}

            def psum(shape, tag):
                return ps.tile(shape, f32, tag=tag, name=tag,
                               bufs=psum_bufs[tag])

            # ---- input DMAs: few, large, on the two hardware-DGE queues
            # (sync/SP and scalar/Activation); gpsimd's software DGE is slow
            xTb = cst.tile([128, XROWS], bf16, tag="xTb")
            nc.sync.dma_start(xTb[:], xTb_in[:, :])
            xfa = cst.tile([128, 4 * CHUNK], bf16, tag="xfa")
            nc.sync.dma_start(xfa[:], xfb_in[:, 0:4 * CHUNK])
            xfb2 = cst.tile([128, 4 * CHUNK], bf16, tag="xfb2")
            nc.sync.dma_start(xfb2[:], xfb_in[:, 4 * CHUNK:T])
            cmb = cst.tile([128, CMB_W], f32, tag="cmb")
            nc.sync.dma_start(cmb[:], cmb_in[:, :])
            # f32 copy of the column-layout x for the DVE chains
            xT = per.tile([128, XROWS], f32, tag="xT")
            nc.scalar.activation(xT[:], xTb[:], ACT.Copy)

            mcat = cmb[:, MCAT_OFF:MCAT_OFF + 2 * RW]
            thrv = cmb[:, THRV_OFF:THRV_OFF + 16]
            v20x = cmb[:, V20X_OFF:V20X_OFF + 16]
            h10x = cmb[:, H10X_OFF:H10X_OFF + 16]
            r10x = cmb[:, R10X_OFF:R10X_OFF + 16]
            ident = cmb[:, ID_OFF:ID_OFF + 128]

            ones = cst.tile([128, 1], f32, tag="ones")
            nc.vector.memset(ones[:], 1.0)
            onesb = cst.tile([128, 1], bf16, tag="onesb")
            nc.vector.memset(onesb[:], 1.0)
            ow20 = cst.tile([128, 1], bf16, tag="ow20")
            nc.vector.memset(ow20[:], float(1.0 / np.sqrt(W20)))
            ow10 = cst.tile([128, 1], bf16, tag="ow10")
            nc.vector.memset(ow10[:], float(1.0 / np.sqrt(W10)))

            out_sb = per.tile([1, OUT_SLOTS], f32, tag="out_sb")
            nc.vector.memset(out_sb[:], 0.0)

            def slot(i):
                return out_sb[:, i:i + 1]

            def psum_scalar(vec_sb, p=128):
                o = psum([1, 1], "sc")
                lhs = ones[0:p, :] if p != 128 else ones[:]
                nc.tensor.matmul(o[:], lhsT=lhs, rhs=vec_sb,
                                 start=True, stop=True, skip_group_check=True)
                return o

            # ============ scalar: squares / signs feeding the chains =========
            x2T = per.tile([128, 1151], f32, tag="x2T")
            nc.scalar.activation(x2T[:], xT[:, 0:1151], ACT.Square)
            sgT = per.tile([128, CHUNK], bf16, tag="sgT")
            nc.scalar.activation(sgT[:], xTb[:, 0:CHUNK], ACT.Sign)

            # ============ rolling sums (S on vector, P=sum x^2 on gpsimd) ====
            # w=10 parts first so its normalizer is ready when cov drains
            s2 = per.tile([128, 1151], f32, tag="Ss2")
            nc.vector.tensor_add(s2[:], xT[:, 0:1151], xT[:, 1:1152])
            s4 = per.tile([128, 1149], f32, tag="Ss4")
            nc.vector.tensor_add(s4[:], s2[:, 0:1149], s2[:, 2:1151])
            s8 = per.tile([128, 1145], f32, tag="Ss8")
            nc.vector.tensor_add(s8[:], s4[:, 0:1145], s4[:, 4:1149])
            S10 = per.tile([128, CHUNK], f32, tag="S10")
            nc.vector.tensor_add(S10[:], s8[:, 0:CHUNK], s2[:, 8:CHUNK + 8])
            s16 = per.tile([128, 1137], f32, tag="Ss16")
            nc.vector.tensor_add(s16[:], s8[:, 0:1137], s8[:, 8:1145])
            S20 = per.tile([128, CHUNK], f32, tag="S20")
            nc.vector.tensor_add(S20[:], s16[:, 0:CHUNK], s4[:, 16:CHUNK + 16])

            p2 = per.tile([128, 1150], f32, tag="Pp2")
            nc.gpsimd.tensor_add(p2[:], x2T[:, 0:1150], x2T[:, 1:1151])
            p4 = per.tile([128, 1148], f32, tag="Pp4")
            nc.gpsimd.tensor_add(p4[:], p2[:, 0:1148], p2[:, 2:1150])
            p8 = per.tile([128, 1144], f32, tag="Pp8")
            nc.gpsimd.tensor_add(p8[:], p4[:, 0:1144], p4[:, 4:1148])
            P10 = per.tile([128, CHUNK], f32, tag="P10")
            nc.gpsimd.tensor_add(P10[:], p8[:, 0:CHUNK], p2[:, 8:CHUNK + 8])

            # d2 = P - S^2/w ; u = 1/sqrt(d2) in bf16 ; us = u*S in bf16
            def make_u(S, P, w, tag):
                ssq_w = per.tile([128, CHUNK], f32, tag=tag + "ssq")
                nc.scalar.activation(ssq_w[:], S[:], ACT.Square,
                                     scale=float(1.0 / np.sqrt(w)))
                d2 = per.tile([128, CHUNK], f32, tag=tag + "d2")
                nc.gpsimd.tensor_tensor(d2[:], P[:], ssq_w[:], ALU.subtract)
                rcp = per.tile([128, CHUNK], f32, tag=tag + "rcp")
                nc.vector.reciprocal_approx_fast(rcp[:], d2[:])
                u = per.tile([128, CHUNK], bf16, tag=tag + "u")
                nc.scalar.activation(u[:], rcp[:], ACT.Sqrt)
                us = per.tile([128, CHUNK], bf16, tag=tag + "us")
                nc.gpsimd.tensor_mul(us[:], u[:], S[:])
                return u, us

            u10, us10 = make_u(S10, P10, W10, "u10")

            p16 = per.tile([128, 1136], f32, tag="Pp16")
            nc.gpsimd.tensor_add(p16[:], p8[:, 0:1136], p8[:, 8:1144])
            P20 = per.tile([128, CHUNK], f32, tag="P20")
            nc.gpsimd.tensor_add(P20[:], p16[:, 0:CHUNK], p4[:, 16:CHUNK + 16])
            u20, us20 = make_u(S20, P20, W20, "u20")

            # ============ tensor: full-T covariance (bf16, replicated) =======
            covp = psum([128, 128], "big")
            for i in range(64):
                src = xfa if i < 32 else xfb2
                o = (i % 32) * 128
                blk = src[:, o:o + 128]
                nc.tensor.matmul(covp[:], lhsT=blk, rhs=blk,
                                 start=(i == 0), stop=(i == 63),
                                 skip_group_check=True)
            cov = per.tile([128, 128], f32, tag="cov")
            nc.vector.tensor_copy(cov[:], covp[:])

            # ============ rolling windows: z matmuls + masked moments ========
            # y[:, 2k] = w20 chunk k, y[:, 2k+1] = w10 chunk k
            r1all = per.tile([128, 16], f32, tag="r1all")
            r2pall = psum([128, 16], "r2p")
            rsqall = per.tile([128, 16], f32, tag="rsqall")

            for k in range(8):
                ksl = slice(k * 128, (k + 1) * 128)
                zp = psum([128, 2 * RW], "zp")
                nc.tensor.matmul(zp[:, 0:RW], lhsT=u20[:, ksl],
                                 rhs=xTb[:, k * 128:k * 128 + RW],
                                 start=True, stop=True, skip_group_check=True)
                nc.tensor.matmul(zp[:, RW:2 * RW], lhsT=u10[:, ksl],
                                 rhs=xTb[:, k * 128:k * 128 + RW],
                                 start=True, stop=True, skip_group_check=True)
                nc.tensor.matmul(r2pall[:, 2 * k:2 * k + 1],
                                 lhsT=us20[:, ksl], rhs=ow20[:],
                                 start=True, stop=True, skip_group_check=True)
                nc.tensor.matmul(r2pall[:, 2 * k + 1:2 * k + 2],
                                 lhsT=us10[:, ksl], rhs=ow10[:],
                                 start=True, stop=True, skip_group_check=True)
                zm = wrk.tile([128, 2 * RW], f32, tag="zm")
                nc.vector.tensor_mul(zm[:], zp[:], mcat)
                V = wrk.tile([128, 2 * RW], f32, tag="V")
                nc.scalar.activation(V[:], zm[:], ACT.Square)
                nc.vector.tensor_reduce(r1all[:, 2 * k:2 * k + 1],
                                        V[:, 0:RW], axis=AX.X, op=ALU.add)
                nc.vector.tensor_reduce(r1all[:, 2 * k + 1:2 * k + 2],
                                        V[:, RW:2 * RW], axis=AX.X,
                                        op=ALU.add)
            nc.scalar.activation(rsqall[:], r2pall[:], ACT.Square)

            # y, finals (batched [128,16])
            y = per.tile([128, 16], f32, tag="y")
            nc.vector.tensor_tensor(y[:], r1all[:], rsqall[:], ALU.subtract)
            c1 = sml.tile([128, 16], f32, tag="c1")
            nc.vector.tensor_tensor(c1[:], y[:], thrv, ALU.is_gt)
            c2 = sml.tile([128, 16], f32, tag="c2")
            nc.vector.tensor_mul(c2[:], c1[:], v20x)
            cnt = sml.tile([128, 1], f32, tag="cnt")
            nc.vector.tensor_reduce(cnt[:], c2[:], axis=AX.X, op=ALU.add)
            nc.vector.tensor_copy(slot(S_COUNT20), psum_scalar(cnt[:])[:])
            hv = sml.tile([128, 16], f32, tag="hv")
            nc.vector.tensor_mul(hv[:], y[:], h10x)
            hs = sml.tile([128, 1], f32, tag="hs")
            nc.vector.tensor_reduce(hs[:], hv[:], axis=AX.X, op=ALU.add)
            nc.vector.tensor_copy(slot(S_HIST10), psum_scalar(hs[:])[:])
            rv = sml.tile([128, 16], f32, tag="rv")
            nc.vector.tensor_mul(rv[:], y[:], r10x)
            rs = sml.tile([128, 1], f32, tag="rs")
            nc.vector.tensor_reduce(rs[:], rv[:], axis=AX.X, op=ALU.add)
            nc.vector.tensor_copy(slot(S_RECENT10), psum_scalar(rs[:])[:])

            # ============ [A,A] post-processing ============
            dscr = wrk.tile([128, 128], f32, tag="dscr")
            nc.vector.tensor_mul(dscr[:], cov[:], ident)
            diag = sml.tile([128, 1], f32, tag="diag")
            nc.vector.tensor_reduce(diag[:], dscr[:], axis=AX.X, op=ALU.add)
            dstd = sml.tile([128, 1], f32, tag="dstd")
            nc.scalar.activation(dstd[:], diag[:], ACT.Sqrt)
            ucol = per.tile([128, 1], f32, tag="ucol")
            nc.vector.reciprocal(ucol[:], dstd[:])
            u2 = sml.tile([128, 1], f32, tag="u2")
            nc.vector.tensor_mul(u2[:], ucol[:], ucol[:])
            du2 = sml.tile([128, 1], f32, tag="du2")
            nc.vector.tensor_mul(du2[:], u2[:], diag[:])
            nc.vector.tensor_copy(slot(S_TRACE), psum_scalar(du2[:])[:])

            uT_p = psum([1, 128], "sc")
            nc.tensor.transpose(uT_p[:], ucol[:], ident)
            uT = per.tile([1, 128], f32, tag="uT")
            nc.vector.tensor_copy(uT[:], uT_p[:])

            def quad_form(mat_sb, out_slot):
                qr = psum([1, 128], "sc")
                nc.tensor.matmul(qr[:], lhsT=ucol[:], rhs=mat_sb,
                                 start=True, stop=True, skip_group_check=True)
                qscr = sml.tile([1, 128], f32, tag="qscr")
                nc.vector.tensor_mul(qscr[:], qr[:], uT[:])
                qacc = sml.tile([1, 1], f32, tag="qacc")
                nc.vector.tensor_reduce(qacc[:], qscr[:], axis=AX.X,
                                        op=ALU.add)
                nc.vector.tensor_copy(out_slot, qacc[:])

            quad_form(cov[:], slot(S_SUMCORR))
            acov = wrk.tile([128, 128], f32, tag="acov")
            nc.scalar.activation(acov[:], cov[:], ACT.Abs)
            quad_form(acov[:], slot(S_SUMABS))

            # corr in bf16 via row-scale, transpose, col-scale
            brow = wrk.tile([128, 128], f32, tag="brow")
            nc.vector.tensor_scalar(brow[:], cov[:], ucol[:], None, ALU.mult)
            bt_p = psum([128, 128], "big")
            nc.tensor.transpose(bt_p[:], brow[:], ident)
            corrb = per.tile([128, 128], bf16, tag="corrb")
            nc.vector.tensor_scalar(corrb[:], bt_p[:], ucol[:], None, ALU.mult)

            # row-layout stats via ones-matmuls, interleaved into the eigen
            # chain's dependency gaps (tensor queue is in-order)
            statp = psum([128, 24], "stat")

            def stat_mms(lo, hi):
                for j in range(lo, hi):
                    nc.tensor.matmul(statp[:, j:j + 1],
                                     lhsT=xTb[:, j * 128:(j + 1) * 128],
                                     rhs=onesb[:], start=True, stop=True,
                                     skip_group_check=True)
                    nc.tensor.matmul(statp[:, 8 + j:9 + j],
                                     lhsT=x2T[:, j * 128:(j + 1) * 128],
                                     rhs=ones[:], start=True, stop=True,
                                     skip_group_check=True)
                    nc.tensor.matmul(statp[:, 16 + j:17 + j],
                                     lhsT=sgT[:, j * 128:(j + 1) * 128],
                                     rhs=onesb[:], start=True, stop=True,
                                     skip_group_check=True)

            # top eigenvalue: 7 bf16 squarings; traces of M^64 and M^128
            M = corrb
            for kk in range(7):
                p = psum([128, 128], "big")
                nc.tensor.matmul(p[:], lhsT=M[:], rhs=M[:],
                                 start=True, stop=True, skip_group_check=True)
                stat_mms(kk, kk + 1)          # fill the cast-wait gap
                if kk >= 5:
                    escr = wrk.tile([128, 128], f32, tag="escr")
                    nc.vector.tensor_mul(escr[:], p[:], ident)
                    edg = sml.tile([128, 1], f32, tag="edg")
                    nc.vector.tensor_reduce(edg[:], escr[:], axis=AX.X,
                                            op=ALU.add)
                    s = S_T6 if kk == 5 else S_T7
                    nc.vector.tensor_copy(slot(s), psum_scalar(edg[:])[:])
                if kk < 6:
                    Mn = wrk.tile([128, 128], bf16, tag="Mn", bufs=2)
                    nc.scalar.activation(Mn[:], p[:], ACT.Copy)
                    M = Mn
            stat_mms(7, 8)

            stats = per.tile([128, 24], f32, tag="stats")
            nc.vector.tensor_copy(stats[:], statp[:])
            sr = stats[:, 0:8]
            ss = stats[:, 8:16]
            rho = stats[:, 16:24]

            # cross-sectional std per t: sqrt((ss - sr^2/A) / (A-1))
            sq2 = sml.tile([128, 8], f32, tag="sq2")
            nc.scalar.activation(sq2[:], sr, ACT.Square,
                                 scale=float(1.0 / np.sqrt(A)))
            varA = sml.tile([128, 8], f32, tag="varA")
            nc.vector.tensor_tensor(varA[:], ss, sq2[:], ALU.subtract)
            csstd = per.tile([128, 8], f32, tag="csstd")
            nc.scalar.activation(csstd[:], varA[:], ACT.Sqrt,
                                 scale=float(1.0 / (A - 1)))
            csr = sml.tile([128, 1], f32, tag="csr")
            nc.vector.tensor_reduce(csr[:], csstd[:], axis=AX.X, op=ALU.add)
            nc.vector.tensor_copy(slot(S_CSSUM), psum_scalar(csr[:])[:])
            nc.vector.tensor_copy(slot(S_CSFIRST), csstd[0:1, 0:1])
            cslast_p = psum([1, 1], "sc")
            nc.tensor.matmul(cslast_p[:], lhsT=cmb[:, OH127_COL:OH127_COL + 1],
                             rhs=csstd[:, 7:8],
                             start=True, stop=True, skip_group_check=True)
            nc.vector.tensor_copy(slot(S_CSLAST), cslast_p[:])

            # ssq partial: sum_t rho_t^2
            rho2 = sml.tile([128, 8], f32, tag="rho2")
            nc.scalar.activation(rho2[:], rho, ACT.Square)
            rr = sml.tile([128, 1], f32, tag="rr")
            nc.vector.tensor_reduce(rr[:], rho2[:], axis=AX.X, op=ALU.add)
            nc.vector.tensor_copy(slot(S_SSQ), psum_scalar(rr[:])[:])

            # ============ position diversity ============
            pa = per.tile([128, 1], f32, tag="pa")
            nc.scalar.activation(pa[:], cmb[:, POS_COL:POS_COL + 1], ACT.Abs)
            nc.vector.tensor_copy(slot(S_PASUM), psum_scalar(pa[:])[:])
            paT_p = psum([1, 128], "sc")
            nc.tensor.transpose(paT_p[:], pa[:], ident)
            paT = sml.tile([1, 128], f32, tag="paT")
            nc.vector.tensor_copy(paT[:], paT_p[:])
            nc.vector.tensor_reduce(slot(S_PAMAX), paT[:], axis=AX.X,
                                    op=ALU.max)

            # ============ herding MLP ============
            h1p = psum([128, 1], "sc")
            nc.tensor.matmul(h1p[:], lhsT=cmb[:, W1A_OFF:W1A_OFF + 128],
                             rhs=cmb[:, XL_COL:XL_COL + 1], start=True,
                             stop=False, skip_group_check=True)
            nc.tensor.matmul(h1p[:], lhsT=cmb[:, W1B_OFF:W1B_OFF + 128],
                             rhs=cmb[:, POS_COL:POS_COL + 1], start=False,
                             stop=True, skip_group_check=True)
            h1 = sml.tile([128, 1], f32, tag="h1")
            nc.scalar.activation(h1[:], h1p[:], ACT.Relu,
                                 bias=cmb[:, B1_COL:B1_COL + 1])
            gk = sml.tile([128, 1], f32, tag="gk")
            nc.vector.tensor_scalar(gk[:], cmb[:, GAM_COL:GAM_COL + 1],
                                    float(1.0 / np.sqrt(1.0 + 1e-5)), None,
                                    ALU.mult)
            h1b = sml.tile([128, 1], f32, tag="h1b")
            nc.vector.tensor_scalar(h1b[:], h1[:], gk[:],
                                    cmb[:, BET_COL:BET_COL + 1],
                                    ALU.mult, ALU.add)
            h2p = psum([64, 1], "sc")
            nc.tensor.matmul(h2p[:], lhsT=cmb[:, W2_OFF:W2_OFF + 64],
                             rhs=h1b[:], start=True, stop=True,
                             skip_group_check=True)
            h2 = sml.tile([64, 1], f32, tag="h2")
            nc.scalar.activation(h2[:], h2p[:], ACT.Relu,
                                 bias=cmb[0:64, B2_COL:B2_COL + 1])
            lg = psum([3, 1], "sc")
            nc.tensor.matmul(lg[:], lhsT=cmb[0:64, W3_OFF:W3_OFF + 3],
                             rhs=h2[:], start=True, stop=True,
                             skip_group_check=True)
            exps = sml.tile([3, 1], f32, tag="exps")
            nc.scalar.activation(exps[:], lg[:], ACT.Exp,
                                 bias=cmb[0:3, B3_COL:B3_COL + 1])
            esum = psum_scalar(exps[:], p=3)
            esum_sb = sml.tile([1, 1], f32, tag="esum_sb")
            nc.vector.tensor_copy(esum_sb[:], esum[:])
            erec = sml.tile([1, 1], f32, tag="erec")
            nc.vector.reciprocal(erec[:], esum_sb[:])
            e2p = psum([1, 1], "sc")
            nc.tensor.matmul(e2p[:], lhsT=cmb[0:3, OH2_COL:OH2_COL + 1],
                             rhs=exps[:], start=True, stop=True,
                             skip_group_check=True)
            e2_sb = sml.tile([1, 1], f32, tag="e2_sb")
            nc.vector.tensor_copy(e2_sb[:], e2p[:])
            nc.vector.tensor_mul(slot(S_SEV), e2_sb[:], erec[:])

            # ============ write out ============
            nc.sync.dma_start(out_d[:, :], out_sb[:])

    nc.compile()
    return nc


def _build_cmb_common(inputs):
    cmb = np.zeros((128, CMB_W), np.float32)
    for j in range(128):
        cmb[j, MCAT_OFF + j:MCAT_OFF + j + W20] = 1.0
        cmb[j, MCAT_OFF + RW + j:MCAT_OFF + RW + j + W10] = 1.0
    cmb[:, THRV_OFF:THRV_OFF + 16:2] = Y_THRESH
    cmb[:, THRV_OFF + 1:THRV_OFF + 16:2] = 1e30
    cmb[:, ID_OFF:ID_OFF + 128] = np.eye(128, dtype=np.float32)
    w1 = np.asarray(inputs["w1"], np.float32)
    cmb[:, W1A_OFF:W1A_OFF + 128] = w1[0:128]
    cmb[:, W1B_OFF:W1B_OFF + 128] = w1[128:256]
    cmb[:, W2_OFF:W2_OFF + 64] = np.asarray(inputs["w2"], np.float32)
    cmb[:, B1_COL] = np.asarray(inputs["b1"], np.float32)
    cmb[:, GAM_COL] = np.asarray(inputs["gamma"], np.float32)
    cmb[:, BET_COL] = np.asarray(inputs["beta"], np.float32)
    cmb[:, POS_COL] = np.asarray(inputs["positions"], np.float32)
    x = np.asarray(inputs["returns_sequence"], np.float32)
    cmb[:, XL_COL] = x[-1]
    cmb[127, OH127_COL] = 1.0
    cmb[0:64, W3_OFF:W3_OFF + 3] = np.asarray(inputs["w3"], np.float32)
    cmb[0:64, B2_COL] = np.asarray(inputs["b2"], np.float32)
    cmb[0:3, B3_COL] = np.asarray(inputs["b3"], np.float32)
    cmb[2, OH2_COL] = 1.0
    return cmb


def _prep_in_maps(inputs):
    import ml_dtypes
    x = np.ascontiguousarray(np.asarray(inputs["returns_sequence"],
                                        dtype=np.float32))
    xfb = np.ascontiguousarray(
        x.reshape(64, 128, 128).transpose(1, 0, 2).reshape(128, T)
        .astype(ml_dtypes.bfloat16))
    cmb_common = _build_cmb_common(inputs)
    in_maps = []
    for c in range(NC_N):
        g = c * CHUNK + np.arange(CHUNK)
        cmb = cmb_common.copy()
        cmb[:, V20X_OFF:V20X_OFF + 16:2] = \
            (g < N20).astype(np.float32).reshape(8, 128).T
        cmb[:, H10X_OFF + 1:H10X_OFF + 16:2] = \
            (g < N10 - 5).astype(np.float32).reshape(8, 128).T
        cmb[:, R10X_OFF + 1:R10X_OFF + 16:2] = \
            ((g >= N10 - 5) & (g < N10)).astype(np.float32).reshape(8, 128).T
        rows = (c * CHUNK + np.arange(XROWS)) % T
        xTc = np.ascontiguousarray(x[rows].T)
        in_maps.append({
            "x_full_b": xfb,
            "xTb_chunk": np.ascontiguousarray(xTc.astype(ml_dtypes.bfloat16)),
            "cmb": np.ascontiguousarray(cmb),
        })
    return in_maps


def _combine(per_core):
    count20 = sum(float(per_core[c][0, S_COUNT20]) for c in range(NC_N))
    hist_y = sum(float(per_core[c][0, S_HIST10]) for c in range(NC_N))
    rec_y = sum(float(per_core[c][0, S_RECENT10]) for c in range(NC_N))
    cs_sum = sum(float(per_core[c][0, S_CSSUM]) for c in range(NC_N))
    ssq_sum = sum(float(per_core[c][0, S_SSQ]) for c in range(NC_N))
    cs_first = float(per_core[0][0, S_CSFIRST])
    cs_last = float(per_core[NC_N - 1][0, S_CSLAST])
    r0 = per_core[0][0]
    sum_corr = float(r0[S_SUMCORR])
    sum_abs = float(r0[S_SUMABS])
    trace_c = float(r0[S_TRACE])
    pa_sum = float(r0[S_PASUM])
    pa_max = float(r0[S_PAMAX])
    severity = float(r0[S_SEV])
    t6, t7 = float(r0[S_T6]), float(r0[S_T7])

    phase_locking = count20 / N20
    hist = (hist_y - A * (N10 - 5)) * INV_OD / (N10 - 5)
    recent = (rec_y - A * 5) * INV_OD / 5.0
    surge = 0.0
    if hist > 0:
        surge = min(max((recent - hist) / hist, 0.0), 1.0)
    avg_disp = cs_sum / T
    trend = -(cs_last - cs_first) / (T - 1)
    herding_index = min(max(trend / (avg_disp + 1e-6) + 0.5, 0.0), 1.0)
    avg_corr = (sum_corr - trace_c) * INV_OD
    lam = (t7 / t6) ** (1.0 / 64.0) if t6 > 0 and t7 > 0 else 1.0
    sync_risk = min(1.0, (lam / A) * avg_corr)
    return_div = 1.0 - sum_abs / (A * A)
    pos_div = 1.0 - pa_max / pa_sum
    div_loss = 1.0 - np.sqrt(return_div * pos_div)
    avg_conc = (A * A / 2.0 + ssq_sum / (2.0 * T) - A) / (A * (A - 1))
    phase_coupling = min(max((avg_conc - 0.5) * 2.0, 0.0), 1.0)
    collective = (herding_index + sync_risk + div_loss) / 3.0
    return np.array([herding_index, severity, sync_risk, phase_locking,
                     div_loss, surge, phase_coupling, collective],
                    dtype=np.float32)


def _ensure_ntff_hook():
    """Install the axon NTFF profile hook if the image lacks antenv.axon_hooks."""
    import sys
    import types
    try:
        import antenv.axon_hooks  # noqa: F401
        return True
    except ImportError:
        pass
    try:
        import antenv
        from trn_agent_boot.trn_boot import _ntff_profile_via_ctypes
        mod = types.ModuleType("antenv.axon_hooks")
        state = {}
        mod.set_axon_ntff_profile_hook = lambda h: state.update(h=h)
        mod.get_axon_ntff_profile_hook = lambda: state.get("h")
        sys.modules["antenv.axon_hooks"] = mod
        antenv.axon_hooks = mod
        hook = _ntff_profile_via_ctypes("/opt/axon/libaxon_pjrt.so")
        mod.set_axon_ntff_profile_hook(hook)
        return hook is not None
    except Exception:
        return False


def _run(inputs, trace=False):
    from concourse.bass_utils import run_bass_kernel_spmd
    if trace:
        trace = _ensure_ntff_hook()
    if "nc" not in _PLAN:
        _PLAN["nc"] = _build_program()
    nc = _PLAN["nc"]
    in_maps = _prep_in_maps(inputs)
    res = run_bass_kernel_spmd(nc, in_maps, core_ids=list(range(NC_N)),
                               trace=trace)
    per_core = [res.results[c]["out_vec"] for c in range(NC_N)]
    return _combine(per_core), res


def kernel(**inputs) -> np.ndarray:
    out, _ = _run(inputs, trace=False)
    return out
